# revision 11
# baseline (speedup 1.0000x reference)
"""Trainium2 Bass kernel for a 2-layer LSTM decoder + vocab projection + log-softmax.

v4 design notes (see git history for the baseline's layout, unchanged here):

- Recurrent matmuls are "flipped": stationary = h k-tile [128k x 32b],
  moving = W_hh (fp8, x2048) in 512-col streams, 4x column-tiled so four
  streams run concurrently on disjoint 32-col PE array groups. Each group
  accumulates in its own PSUM bank (HW requires one start=True chain per
  bank). Blocks are copied to SBUF and transposed back to the [128q, m*b]
  gate layout with small PE matmuls against a selection matrix S whose
  entries are 1/2048 - folding the fp8 weight scale away for free.

- Ticks are ordered so the PE queue never drains (HAM throttles the PE
  clock to 1.2 GHz after idle windows): projection matmul slices are
  interleaved between dependent recurrence segments, softmax stats run at
  tick end, and the layer-1 input pass borrows the flip-pool PSUM banks.

- W_out is fully resident as fp8 (x2048). All four vocab quarters of each
  token group, the z = sum(exp) AllReduce (replaces separate max+sum ARs),
  and the final logp = logits/2048 - logZ subtraction all run inside the
  recurrence; the tail handles only the last groups. Output is bf16.
"""

import numpy as np
import ml_dtypes
from contextlib import ExitStack

import concourse.bass as bass
import concourse.mybir as mybir
import concourse.tile as tile
from concourse import bacc
from concourse import bass_utils

F32 = mybir.dt.float32
BF16 = mybir.dt.bfloat16
FP8 = mybir.dt.float8e4
I32 = mybir.dt.int32
AF = mybir.ActivationFunctionType
bf16 = ml_dtypes.bfloat16
f8 = ml_dtypes.float8_e4m3fn

H = 1024
RH = 2048
V = 32000
B = 32
T = 128
NC = 8
GS = 1024          # gate rows per core per layer
HS = 256           # h dims per core
VS = V // NC       # vocab rows per core
VQ = VS // 4       # vocab quarter (1000)
D = 8              # layer-1 input-matmul batching (steps per weight pass)
LAG = D + 1        # layer-1 step lag behind layer 0
SOS_ID = 1
import os
OS = 2048.0        # fp8 weight scale
INREC_Z = os.environ.get("KV_INREC_Z", "0") == "1"
PROJ_INREC = os.environ.get("KV_PROJ_INREC", "1") == "1"
L1IN_FLP = os.environ.get("KV_L1IN_FLP", "1") == "1"
PROJ_SPLIT = os.environ.get("KV_PROJ_SPLIT", "0") == "1"

TS_FULL = T - 1    # 127 recurrence steps


def _p_major(w, kt, mt):
    """(kt*128, mt*128) -> (128, kt*mt*128) packed [p, k*mt*128 + m*128 + q]."""
    return np.ascontiguousarray(
        w.reshape(kt, 128, mt, 128).transpose(1, 0, 2, 3).reshape(128, kt * mt * 128)
    )


def _ktile_cols(a):
    """(kt*128, n) -> (128, kt*n) packed [p, k*n + j] = a[128k+p, j]."""
    kt = a.shape[0] // 128
    return np.ascontiguousarray(
        a.reshape(kt, 128, a.shape[1]).transpose(1, 0, 2).reshape(128, kt * a.shape[1])
    )


def prep_inputs(inp, ts=TS_FULL):
    """Host-side prep: slice/transpose/cast weights per core -> in_maps."""
    ntok_pad = ((ts * B + 127) // 128) * 128
    f32 = np.float32

    emb = np.asarray(inp["emb"], f32)
    tb = np.asarray(inp["target_batch"]).astype(np.int64)
    idx = tb[:, :ts].T.reshape(-1).astype(np.int32)       # (ts*B,) t-major
    idx = np.concatenate([idx, np.zeros(ntok_pad - idx.size, np.int32)])
    idx = np.ascontiguousarray(idx.reshape(ntok_pad // 128, 128).T)  # [p, group]

    ch = np.asarray(inp["context_h"], f32)
    cc = np.asarray(inp["context_c"], f32)
    h_init = np.concatenate([ch[0::2], ch[1::2]], axis=2)  # (2, B, RH)
    c_init = np.concatenate([cc[0::2], cc[1::2]], axis=2)

    def h_pack(hl):  # (B, RH) -> (128, 512) bf16 [p, 32k+b] = h[b, 128k+p]
        return np.ascontiguousarray(
            hl.T.reshape(16, 128, B).transpose(1, 0, 2).reshape(128, 16 * B)
        ).astype(bf16)

    Wih = [np.asarray(inp["W_ih0"], f32), np.asarray(inp["W_ih1"], f32)]
    Whh = [np.asarray(inp["W_hh0"], f32), np.asarray(inp["W_hh1"], f32)]
    bsum = [np.asarray(inp["b_ih0"], f32) + np.asarray(inp["b_hh0"], f32),
            np.asarray(inp["b_ih1"], f32) + np.asarray(inp["b_hh1"], f32)]
    W_out = np.asarray(inp["W_out"], f32)
    b_out = np.asarray(inp["b_out"], f32)

    # selection matrix with the 1/OS descale folded in
    S = np.zeros((128, B), np.float32)
    for g in range(4):
        S[32 * g + np.arange(B), np.arange(B)] = 1.0 / OS
    S = S.astype(bf16)

    in_maps = []
    for c in range(NC):
        # gate rows for core c, in i,f,o,g chunk order (256 rows each)
        rows = np.concatenate([np.arange(RH * k + HS * c, RH * k + HS * (c + 1))
                               for k in (0, 1, 3, 2)])  # i,f,o,g
        wih0t = _p_major(Wih[0][rows].T.astype(bf16), 8, 8)       # (128, 8192)
        whh0t = _p_major((Whh[0][rows].T * OS).astype(f8), 16, 8)  # fp8
        wih1t = _p_major((Wih[1][rows].T * OS).astype(f8), 16, 8)
        whh1t = _p_major((Whh[1][rows].T * OS).astype(f8), 16, 8)
        b0 = np.ascontiguousarray(bsum[0][rows].reshape(8, 128).T)  # (128, 8)
        b1 = np.ascontiguousarray(bsum[1][rows].reshape(8, 128).T)
        woutt = _ktile_cols(
            (W_out[VS * c:VS * (c + 1)].T * OS).astype(f8))   # (128, 64000) fp8
        boutc = (b_out[VS * c:VS * (c + 1)] * OS).reshape(1, VS).astype(bf16)

        def c_pack(cl):  # (B, RH) slice -> (128, 64) f32
            s = cl[:, HS * c:HS * (c + 1)].T  # (256, B)
            return np.ascontiguousarray(
                s.reshape(2, 128, B).transpose(1, 0, 2).reshape(128, 2 * B))

        in_maps.append({
            "idx": idx, "embt": emb, "smat": S,
            "wih0t": wih0t, "whh0t": whh0t, "wih1t": wih1t, "whh1t": whh1t,
            "b0": b0, "b1": b1, "woutt": woutt, "boutc": boutc,
            "h0init": h_pack(h_init[0]), "h1init": h_pack(h_init[1]),
            "c0init": c_pack(c_init[0]), "c1init": c_pack(c_init[1]),
        })
    return in_maps, ntok_pad


def build_nc(ts=TS_FULL):
    ntok_pad = ((ts * B + 127) // 128) * 128
    ntok = ts * B
    ngrp = ntok_pad // 128          # token groups of 128 for projection
    nticks = ts + LAG + 1

    nc = bacc.Bacc("TRN2", target_bir_lowering=False, debug=False,
                   enable_asserts=False, num_devices=NC)

    # ---- I/O ----
    idx_t = nc.dram_tensor("idx", [128, ntok_pad // 128], I32,
                           kind="ExternalInput").ap()
    emb_t = nc.dram_tensor("embt", [V, H], F32, kind="ExternalInput").ap()
    s_t = nc.dram_tensor("smat", [128, B], BF16, kind="ExternalInput").ap()
    wih0_t = nc.dram_tensor("wih0t", [128, 8 * GS], BF16, kind="ExternalInput").ap()
    whh0_t = nc.dram_tensor("whh0t", [128, 16 * GS], FP8, kind="ExternalInput").ap()
    wih1_t = nc.dram_tensor("wih1t", [128, 16 * GS], FP8, kind="ExternalInput").ap()
    whh1_t = nc.dram_tensor("whh1t", [128, 16 * GS], FP8, kind="ExternalInput").ap()
    b0_t = nc.dram_tensor("b0", [128, 8], F32, kind="ExternalInput").ap()
    b1_t = nc.dram_tensor("b1", [128, 8], F32, kind="ExternalInput").ap()
    wout_t = nc.dram_tensor("woutt", [128, 16 * VS], FP8, kind="ExternalInput").ap()
    bout_t = nc.dram_tensor("boutc", [1, VS], BF16, kind="ExternalInput").ap()
    h0i_t = nc.dram_tensor("h0init", [128, 512], BF16, kind="ExternalInput").ap()
    h1i_t = nc.dram_tensor("h1init", [128, 512], BF16, kind="ExternalInput").ap()
    c0i_t = nc.dram_tensor("c0init", [128, 64], F32, kind="ExternalInput").ap()
    c1i_t = nc.dram_tensor("c1init", [128, 64], F32, kind="ExternalInput").ap()
    out_t = nc.dram_tensor("out", [ntok_pad, VS], BF16, kind="ExternalOutput").ap()

    RG = [list(range(NC))]
    PCH = [(0, 512), (512, VQ - 512)]   # vocab chunk split per quarter

    with ExitStack() as ctx:
        tc = ctx.enter_context(tile.TileContext(nc))
        dram = ctx.enter_context(tc.tile_pool(name="dram", bufs=1, space="DRAM"))
        agp = ctx.enter_context(tc.tile_pool(name="agp", bufs=6, space="DRAM"))
        keep = ctx.enter_context(tc.tile_pool(name="keep", bufs=1))

        # long-lived: softmax stats, resident fp8 W_out, consts
        m4 = keep.tile([128, 4 * ngrp], F32, tag="m4")
        s4 = keep.tile([128, 4 * ngrp], F32, tag="s4")
        logZ = keep.tile([128, ngrp], F32, tag="logZ")
        ones_s = keep.tile([1, 128], BF16, tag="ones")
        bout_s = keep.tile([1, VS], BF16, tag="bouts")
        s_s = keep.tile([128, B], BF16, tag="ss")
        wout_s = keep.tile([128, 16 * VS], FP8, tag="wouts")

        # persistent DRAM
        xbf_d = dram.tile([ntok_pad, H], BF16, tag="xbf")
        g0_d = dram.tile([8, 128, ntok], BF16, tag="g0d")
        outs_d = dram.tile([ngrp, 16, 128, 128], BF16, tag="outsd")
        logits_d = dram.tile([ngrp, 128, VS], BF16, tag="logitsd")

        nc.gpsimd.memset(ones_s[:], 1.0)
        nc.sync.dma_start(bout_s[:], bout_t[:])
        nc.sync.dma_start(s_s[:], s_t[:])

        def proj_mms(g, q, osb, psq, kts, with_bias):
            """Projection matmul slice: k-tiles `kts` of quarter q."""
            v0 = VQ * q
            for k in kts:
                lhs = osb[:, 128 * k:128 * (k + 1)]
                for (o, w) in PCH:
                    nc.tensor.matmul(
                        psq[:, o:o + w], lhs,
                        wout_s[:, k * VS + v0 + o: k * VS + v0 + o + w],
                        start=(k == 0), stop=False)
            if with_bias:
                for (o, w) in PCH:
                    nc.tensor.matmul(psq[:, o:o + w], ones_s[:, :],
                                     bout_s[:, v0 + o:v0 + o + w],
                                     start=False, stop=True)

        def proj_stats(g, q, psq, scr_pool):
            gh = 4 * g + q
            v0 = VQ * q
            nc.vector.tensor_reduce(m4[:, gh:gh + 1], psq[:, :VQ],
                                    axis=mybir.AxisListType.X,
                                    op=mybir.AluOpType.max)
            negm = scr_pool.tile([128, 1], F32, tag="negm", name=f"nm{g}_{q}")
            nc.vector.tensor_scalar_mul(negm[:], m4[:, gh:gh + 1], -1.0 / OS)
            esc = scr_pool.tile([128, VQ], BF16, tag="esc", name=f"esc{g}_{q}")
            nc.scalar.activation(esc[:], psq[:, :VQ], AF.Exp,
                                 bias=negm[:, :1], scale=1.0 / OS,
                                 accum_out=s4[:, gh:gh + 1])
            lsb = scr_pool.tile([128, VQ], BF16, tag="lsb", name=f"lsb{g}_{q}")
            nc.vector.tensor_copy(lsb[:], psq[:, :VQ])
            nc.scalar.dma_start(logits_d[g, :, v0:v0 + VQ], lsb[:])

        # ============ Phase 0: embeddings + G0 = X @ Wih0.T + b0 ============
        with tc.tile_pool(name="rp", bufs=1) as rp:
            whh0_s = rp.tile([128, 16 * GS], FP8, tag="whh0s")
            b0_s = rp.tile([128, 8], F32, tag="b0s")
            b1_s = rp.tile([128, 8], F32, tag="b1s")
            h0ring = rp.tile([128, 16 * 512], BF16, tag="h0ring")   # 16 slots
            h1ring = rp.tile([128, 2 * 512], BF16, tag="h1ring")    # 2 slots
            g0ring = rp.tile([128, 2 * 2048], BF16, tag="g0ring")   # 2 x 8 steps
            g1ring = rp.tile([128, 2048], BF16, tag="g1ring")       # D steps [m,s,b]

            nc.sync.dma_start(whh0_s[:], whh0_t[:])
            nc.sync.dma_start(b0_s[:], b0_t[:])
            nc.sync.dma_start(b1_s[:], b1_t[:])
            # initial h into the ring slots read at t=0 / j=0
            nc.sync.dma_start(h0ring[:, 15 * 512:16 * 512], h0i_t[:])
            nc.sync.dma_start(h1ring[:, 1 * 512:2 * 512], h1i_t[:])

            TH = 2048  # token half for XT chunking
            with tc.tile_pool(name="p0sb", bufs=2) as p0sb, \
                 tc.tile_pool(name="p0ev", bufs=2) as p0ev, \
                 tc.tile_pool(name="p0big", bufs=1) as p0big, \
                 tc.tile_pool(name="p0ps", bufs=2, space="PSUM") as p0ps:
                idxs = p0big.tile([128, ntok_pad // 128], I32, tag="idxs")
                nc.sync.dma_start(idxs[:], idx_t[:])
                zpad = p0big.tile([128, 512], BF16, tag="zpad")
                nc.gpsimd.memset(zpad[:], 0.0)
                nc.scalar.dma_start(
                    outs_d[ngrp - 1, :, :, 96:128].rearrange("k p b -> p k b"),
                    zpad[:].rearrange("p (k b) -> p k b", k=16))
                for it in range(ntok_pad // 128):
                    xg = p0sb.tile([128, H], F32, tag="xg")
                    nc.gpsimd.indirect_dma_start(
                        out=xg[:], out_offset=None, in_=emb_t[:],
                        in_offset=bass.IndirectOffsetOnAxis(
                            ap=idxs[:, it:it + 1], axis=0))
                    xc = p0sb.tile([128, H], BF16, tag="xc")
                    nc.vector.tensor_copy(xc[:], xg[:])
                    nc.scalar.dma_start(xbf_d[128 * it:128 * (it + 1), :], xc[:])

                wih0_s = p0big.tile([128, 8 * GS], BF16, tag="wih0s")
                nc.sync.dma_start(wih0_s[:], wih0_t[:])
                xt_s = p0big.tile([128, 8 * TH], BF16, tag="xts")

                for half in range((ntok + TH - 1) // TH):
                    t0 = TH * half
                    tw = min(TH, ntok - t0)
                    twp = ((tw + 15) // 16) * 16  # transpose src rows mult of 16
                    for k in range(8):
                        nc.sync.dma_start_transpose(
                            xt_s[:, TH * k:TH * k + twp],
                            xbf_d[t0:t0 + twp, 128 * k:128 * (k + 1)])
                    nch = [(512 * i, min(512, tw - 512 * i))
                           for i in range((tw + 511) // 512)]
                    for m in range(8):
                        ps = p0ps.tile([128, 2048], F32, tag="p0ps")
                        for k in range(8):
                            lhs = wih0_s[:, k * GS + 128 * m: k * GS + 128 * (m + 1)]
                            for (o, w) in nch:
                                nc.tensor.matmul(
                                    ps[:, o:o + w], lhs,
                                    xt_s[:, TH * k + o: TH * k + o + w],
                                    start=(k == 0), stop=(k == 7))
                        ev = p0ev.tile([128, TH], BF16, tag="g0ev")
                        nc.scalar.activation(ev[:, :tw], ps[:, :tw], AF.Identity,
                                             bias=b0_s[:, m:m + 1])
                        nc.scalar.dma_start(g0_d[m, :, t0:t0 + tw], ev[:, :tw])

            # ============ Phase 1: recurrence ============
            c_prev = [None, None]
            done_quarters = set()
            done_zb = set()
            done_pb = set()
            with tc.tile_pool(name="rp2", bufs=1) as rp2, \
                 tc.tile_pool(name="flp", bufs=1, space="PSUM") as flp, \
                 tc.tile_pool(name="gps", bufs=1, space="PSUM") as gps, \
                 tc.tile_pool(name="shp", bufs=1, space="PSUM") as shp, \
                 tc.tile_pool(name="gsbp", bufs=2) as gsbp, \
                 tc.tile_pool(name="posb", bufs=1) as posb_pool, \
                 tc.tile_pool(name="pscr", bufs=1) as pscr_pool, \
                 tc.tile_pool(name="pbp", bufs=2) as pbp, \
                 tc.tile_pool(name="zp", bufs=1) as zp, \
                 tc.tile_pool(name="cell", bufs=2) as cell_pool:

                wih1_s = rp2.tile([128, 16 * GS], FP8, tag="wih1s")
                whh1_s = rp2.tile([128, 16 * GS], FP8, tag="whh1s")
                nc.scalar.dma_start(wih1_s[:], wih1_t[:])
                nc.scalar.dma_start(whh1_s[:], whh1_t[:])
                nc.scalar.dma_start(wout_s[:], wout_t[:])

                def g0_prefetch(blk):
                    t0 = 8 * blk
                    nsteps = min(8, ts - t0)
                    if nsteps <= 0:
                        return
                    dst = g0ring[:].rearrange("p (h m s b) -> p h m s b",
                                              h=2, m=8, b=B)
                    src = g0_d[:, :, B * t0: B * (t0 + nsteps)].rearrange(
                        "m p sb -> p m sb")
                    nc.scalar.dma_start(
                        dst[:, blk % 2, :, 0:nsteps, :].rearrange(
                            "p m s b -> p m (s b)"), src)

                def flip_mms(w_s, h_of_kt, li):
                    """Flipped hh matmul into 4 one-bank psum tiles."""
                    Ps = [flp.tile([128, 512], F32, tag=f"fl{g}",
                                   name=f"fl{g}_{li}") for g in range(4)]
                    for r in range(8):
                        for g in range(4):
                            jm, jk = g >> 1, g & 1
                            kt = 8 * jk + r
                            nc.tensor.matmul(
                                Ps[g][32 * g:32 * (g + 1), :],
                                h_of_kt(kt),
                                w_s[:, kt * GS + 512 * jm: kt * GS + 512 * jm + 512],
                                start=(r == 0), stop=(r == 7),
                                tile_position=(0, 32 * g))
                    gsb = gsbp.tile([128, 1024], BF16, tag="gsb",
                                    name=f"gsb{li}")
                    for g in range(4):
                        jm = g >> 1
                        dst = gsb[32 * g:32 * (g + 1), 512 * jm:512 * jm + 512]
                        src = Ps[g][32 * g:32 * (g + 1), :]
                        if g % 2 == 0:
                            nc.scalar.activation(dst, src, AF.Identity)
                        else:
                            nc.vector.tensor_copy(dst, src)
                    return gsb

                def combine(gsb, li):
                    """Transpose-combine gsb blocks -> G [128q, 8m x 32b]."""
                    G = gps.tile([128, 512], F32, tag=f"G{li}", name=f"G{li}")
                    for cch in range(8):
                        jm = cch // 4
                        nc.tensor.matmul(
                            G[:, B * cch:B * (cch + 1)],
                            gsb[64 * jm:64 * jm + 64, 128 * cch:128 * (cch + 1)],
                            s_s[64 * jm:64 * jm + 64, :],
                            start=True, stop=True)
                    return G

                def cell(l, G, gadd_ap):
                    """LSTM cell for layer l; returns hn tile [128, 64] bf16."""
                    g = cell_pool.tile([128, 256], F32, tag=f"g{l}")
                    nc.vector.tensor_add(
                        g[:].rearrange("p (m b) -> p m b", b=B),
                        G[:, :256].rearrange("p (m b) -> p m b", b=B),
                        gadd_ap)
                    sfo = cell_pool.tile([128, 192], F32, tag=f"sfo{l}")
                    nc.scalar.activation(sfo[:], g[:, 0:192], AF.Sigmoid)
                    tg = cell_pool.tile([128, 64], F32, tag=f"tg{l}")
                    nc.scalar.activation(tg[:], g[:, 192:256], AF.Tanh)
                    t1 = cell_pool.tile([128, 64], F32, tag=f"t1{l}")
                    nc.vector.tensor_mul(t1[:], sfo[:, 0:64], tg[:])
                    t2 = cell_pool.tile([128, 64], F32, tag=f"t2{l}")
                    nc.vector.tensor_mul(t2[:], sfo[:, 64:128], c_prev[l][:])
                    cn = cell_pool.tile([128, 64], F32, tag=f"cn{l}")
                    nc.vector.tensor_add(cn[:], t1[:], t2[:])
                    c_prev[l] = cn
                    tcn = cell_pool.tile([128, 64], F32, tag=f"tc{l}")
                    nc.scalar.activation(tcn[:], cn[:], AF.Tanh)
                    hn = cell_pool.tile([128, 64], BF16, tag=f"hn{l}")
                    nc.vector.tensor_mul(hn[:], sfo[:, 128:192], tcn[:])
                    return hn

                def zbatch(b):
                    """z-AllReduce + logZ for groups 4b..4b+3."""
                    gsl = slice(4 * b, 4 * b + 4)
                    m4v = m4[:].rearrange("p (g q) -> p g q", q=4)[:, gsl, :]
                    s4v = s4[:].rearrange("p (g q) -> p g q", q=4)[:, gsl, :]
                    t01 = zp.tile([128, 4], F32, tag="t01", name=f"t01_{b}")
                    t23 = zp.tile([128, 4], F32, tag="t23", name=f"t23_{b}")
                    nc.vector.tensor_max(t01[:], m4v[:, :, 0], m4v[:, :, 1])
                    nc.vector.tensor_max(t23[:], m4v[:, :, 2], m4v[:, :, 3])
                    mall = zp.tile([128, 4], F32, tag="mall", name=f"ma{b}")
                    nc.vector.tensor_max(mall[:], t01[:], t23[:])
                    acc = zp.tile([128, 4], F32, tag="acc", name=f"ac{b}")
                    dq = zp.tile([128, 4], F32, tag="dq", name=f"dq{b}")
                    for q in range(4):
                        nc.vector.tensor_sub(dq[:], m4v[:, :, q], mall[:])
                        nc.scalar.activation(dq[:], dq[:], AF.Exp, scale=1.0 / OS)
                        nc.vector.tensor_mul(dq[:], dq[:], s4v[:, :, q])
                        if q == 0:
                            nc.vector.tensor_copy(acc[:], dq[:])
                        else:
                            nc.vector.tensor_add(acc[:], acc[:], dq[:])
                    em = zp.tile([128, 4], F32, tag="em", name=f"em{b}")
                    nc.scalar.activation(em[:], mall[:], AF.Exp, scale=1.0 / OS)
                    zt = zp.tile([128, 4], F32, tag="zt", name=f"zt{b}")
                    nc.vector.tensor_mul(zt[:], acc[:], em[:])
                    zloc = agp.tile([128, 4], F32, tag="zloc", name=f"zl{b}")
                    zglob = agp.tile([128, 4], F32, tag="zglob",
                                     name=f"zg{b}", addr_space="Shared")
                    nc.sync.dma_start(zloc[:], zt[:])
                    nc.gpsimd.collective_compute(
                        "AllReduce", mybir.AluOpType.add, replica_groups=RG,
                        ins=[zloc[:].opt()], outs=[zglob[:].opt()])
                    zg_s = zp.tile([128, 4], F32, tag="zgs", name=f"zs{b}")
                    nc.sync.dma_start(zg_s[:], zglob[:])
                    nc.scalar.activation(logZ[:, gsl], zg_s[:], AF.Ln)
                    done_zb.add(b)

                def passb_chunk(g, ci):
                    """logp chunk: out[128g.., 1000ci..] = lin/OS - logZ[g]."""
                    v0 = VQ * ci
                    lin = pbp.tile([128, VQ], BF16, tag="lin",
                                   name=f"li{g}_{ci}")
                    nc.sync.dma_start(lin[:], logits_d[g, :, v0:v0 + VQ])
                    lout = pbp.tile([128, VQ], BF16, tag="lout",
                                    name=f"lo{g}_{ci}")
                    nc.vector.tensor_scalar(lout[:], lin[:], 1.0 / OS,
                                            logZ[:, g:g + 1],
                                            op0=mybir.AluOpType.mult,
                                            op1=mybir.AluOpType.subtract)
                    nc.sync.dma_start(
                        out_t[128 * g:128 * (g + 1), v0:v0 + VQ], lout[:])
                    done_pb.add((g, ci))

                c0s = cell_pool.tile([128, 64], F32, tag="cn0")
                nc.sync.dma_start(c0s[:], c0i_t[:])
                c_prev[0] = c0s
                c1s = cell_pool.tile([128, 64], F32, tag="cn1")
                nc.sync.dma_start(c1s[:], c1i_t[:])
                c_prev[1] = c1s

                g0_prefetch(0)
                g0_prefetch(1)

                h0r4 = h0ring[:].rearrange("p (s k b) -> p s k b", s=16, b=B)
                h1r4 = h1ring[:].rearrange("p (s k b) -> p s k b", s=2, b=B)
                g0r5 = g0ring[:].rearrange("p (h m s b) -> p h m s b",
                                           h=2, m=8, b=B)
                g1r4 = g1ring[:].rearrange("p (m s b) -> p m s b", m=8, b=B)

                # schedules: one projection quarter per tick; z-AR per 4
                # groups; one passB chunk per tick after logZ is known
                psched = {}
                if PROJ_INREC:
                    for g in range(ngrp - 1):
                        for q in range(4):
                            t_q = 4 * g + 14 + q
                            if t_q < nticks:
                                psched[t_q] = (g, q)
                zsched = {}
                pbsched = {}
                if INREC_Z:
                    for b2 in range((ngrp + 3) // 4):
                        tz = 16 * b2 + 31
                        if tz < nticks:
                            zsched[tz] = b2
                        for i in range(16):
                            tp = 16 * b2 + 32 + i
                            if tp < nticks:
                                pbsched[tp] = (4 * b2 + i // 4, i % 4)
                cur_osb = [None]

                # one merged AllGather per tick: hn0(t) + hn1 of last tick
                agin_next = agp.tile([2, 2, 128, B], BF16, tag="agin")
                for t in range(nticks):
                    j = t - LAG  # layer-1 step this tick
                    agin = agin_next
                    agin_next = agp.tile([2, 2, 128, B], BF16, tag="agin")

                    pj = psched.get(t)
                    psq = None
                    if pj is not None:
                        g, q = pj
                        if q == 0:
                            osb = posb_pool.tile([128, 2048], BF16, tag="posb",
                                                 name=f"osb{g}")
                            nc.sync.dma_start(
                                osb[:].rearrange("p (k q) -> p k q", k=16),
                                outs_d[g, :, :, :].rearrange("k p q -> p k q"))
                            cur_osb[0] = osb
                        psq = shp.tile([128, 1024], F32, tag="sh",
                                       name=f"psq{g}_{q}")

                    # ---- layer 0, step t ----
                    if t < ts:
                        gsb0 = flip_mms(
                            whh0_s,
                            lambda kt: h0r4[:, (t - 1) % 16, kt, :], "a")
                    # proj slice 1 (PE filler while gsb0 copies run)
                    if pj is not None and PROJ_SPLIT:
                        proj_mms(g, q, cur_osb[0], psq, range(0, 6), False)
                    if t < ts:
                        G0 = combine(gsb0, "a")
                        if t % 8 == 7:
                            g0_prefetch(t // 8 + 2)
                        hn0 = cell(0, G0, g0r5[:, (t // 8) % 2, :, t % 8, :])
                        nc.sync.dma_start(
                            agin[:, 0, :, :].rearrange("j p b -> p j b"),
                            hn0[:].rearrange("p (j b) -> p j b", b=B))

                    agout = agp.tile([NC, 2, 2, 128, B], BF16, tag="agout",
                                     addr_space="Shared")
                    nc.gpsimd.collective_compute(
                        "AllGather", mybir.AluOpType.bypass, replica_groups=RG,
                        ins=[agin[:].opt()], outs=[agout[:].opt()])
                    if t < ts:
                        nc.sync.dma_start(
                            h0r4[:, t % 16, :, :],
                            agout[:, :, 0].rearrange("r j p b -> p (r j) b"))
                    jj = t - 1 - LAG  # step whose h1 rides this AG
                    if 0 <= jj < ts:
                        nc.sync.dma_start(
                            h1r4[:, jj % 2, :, :],
                            agout[:, :, 1].rearrange("r j p b -> p (r j) b"))
                        nc.scalar.dma_start(
                            outs_d[jj // 4, :, :, B * (jj % 4):B * (jj % 4 + 1)]
                            .rearrange("k p b -> p k b"),
                            h1r4[:, jj % 2, :, :])

                    # ---- layer-1 input pass every D steps ----
                    # four quarter-passes through the flip-pool banks
                    if 0 <= j < ts and j % D == 0:
                        nb = min(D, ts - j)
                        s0 = j % 16
                        if L1IN_FLP:
                            npass, mper, ptag = 4, 2, None
                        else:
                            npass, mper, ptag = 2, 4, "sh"
                        for qi in range(npass):
                            if L1IN_FLP:
                                psg = flp.tile([128, 512], F32, tag=f"fl{qi}",
                                               name=f"psg{t}_{qi}")
                            else:
                                psg = shp.tile([128, 1024], F32, tag="sh",
                                               name=f"psg{t}_{qi}")
                            for lmi in range(mper):
                                m = mper * qi + lmi
                                for k in range(16):
                                    nc.tensor.matmul(
                                        psg[:, 256 * lmi: 256 * lmi + B * nb],
                                        wih1_s[:, k * GS + 128 * m:
                                               k * GS + 128 * (m + 1)],
                                        h0r4[:, s0:s0 + nb, k, :],
                                        start=(k == 0), stop=(k == 15))
                            for lmi in range(mper):
                                m = mper * qi + lmi
                                nc.scalar.activation(
                                    g1ring[:, 256 * m: 256 * m + B * nb],
                                    psg[:, 256 * lmi: 256 * lmi + B * nb],
                                    AF.Identity, scale=1.0 / OS,
                                    bias=b1_s[:, m:m + 1])

                    # proj slice 2
                    if pj is not None and PROJ_SPLIT:
                        proj_mms(g, q, cur_osb[0], psq, range(6, 12), False)

                    # ---- layer 1, step j ----
                    if 0 <= j < ts:
                        gsb1 = flip_mms(
                            whh1_s,
                            lambda kt: h1r4[:, (j - 1) % 2, kt, :], "b")
                    # proj slice 3 (filler while gsb1 copies run)
                    if pj is not None and PROJ_SPLIT:
                        proj_mms(g, q, cur_osb[0], psq, range(12, 16), True)
                    if 0 <= j < ts:
                        G1 = combine(gsb1, "b")
                        hn1 = cell(1, G1, g1r4[:, :, j % D, :])
                        nc.sync.dma_start(
                            agin_next[:, 1, :, :].rearrange("j p b -> p j b"),
                            hn1[:].rearrange("p (j b) -> p j b", b=B))

                    # stats at tick end (keeps DVE/ACT off the PE's path)
                    if pj is not None:
                        if not PROJ_SPLIT:
                            proj_mms(g, q, cur_osb[0], psq, range(16), True)
                        proj_stats(g, q, psq, pscr_pool)
                        done_quarters.add((g, q))
                    if t in zsched:
                        zbatch(zsched[t])
                    if t in pbsched:
                        gpb, ci = pbsched[t]
                        passb_chunk(gpb, ci)

        # ============ Phase 2 tail: leftovers ============
        with tc.tile_pool(name="p2sb", bufs=2) as p2sb, \
             tc.tile_pool(name="p2scr", bufs=2) as p2scr, \
             tc.tile_pool(name="p2z", bufs=1) as p2z, \
             tc.tile_pool(name="p2pb", bufs=2) as p2pb, \
             tc.tile_pool(name="p2ps", bufs=2, space="PSUM") as p2ps:

            # remaining projection quarters
            for g in range(ngrp):
                rem = [q for q in range(4) if (g, q) not in done_quarters]
                if not rem:
                    continue
                osb = p2sb.tile([128, 2048], BF16, tag="osb", name=f"osbt{g}")
                nc.sync.dma_start(
                    osb[:].rearrange("p (k q) -> p k q", k=16),
                    outs_d[g, :, :, :].rearrange("k p q -> p k q"))
                for q in rem:
                    psq = p2ps.tile([128, 1024], F32, tag="sh",
                                    name=f"tps{g}_{q}")
                    proj_mms(g, q, osb, psq, range(16), True)
                    proj_stats(g, q, psq, p2scr)

            # remaining z batches (reuse the in-rec helper pools shapes)
            nb2 = (ngrp + 3) // 4
            for b2 in range(nb2):
                if b2 in done_zb:
                    continue
                gsl = slice(4 * b2, 4 * b2 + 4)
                m4v = m4[:].rearrange("p (g q) -> p g q", q=4)[:, gsl, :]
                s4v = s4[:].rearrange("p (g q) -> p g q", q=4)[:, gsl, :]
                t01 = p2z.tile([128, 4], F32, tag="t01", name=f"u01_{b2}")
                t23 = p2z.tile([128, 4], F32, tag="t23", name=f"u23_{b2}")
                nc.vector.tensor_max(t01[:], m4v[:, :, 0], m4v[:, :, 1])
                nc.vector.tensor_max(t23[:], m4v[:, :, 2], m4v[:, :, 3])
                mall = p2z.tile([128, 4], F32, tag="mall", name=f"uma{b2}")
                nc.vector.tensor_max(mall[:], t01[:], t23[:])
                acc = p2z.tile([128, 4], F32, tag="acc", name=f"uac{b2}")
                dq = p2z.tile([128, 4], F32, tag="dq", name=f"udq{b2}")
                for q in range(4):
                    nc.vector.tensor_sub(dq[:], m4v[:, :, q], mall[:])
                    nc.scalar.activation(dq[:], dq[:], AF.Exp, scale=1.0 / OS)
                    nc.vector.tensor_mul(dq[:], dq[:], s4v[:, :, q])
                    if q == 0:
                        nc.vector.tensor_copy(acc[:], dq[:])
                    else:
                        nc.vector.tensor_add(acc[:], acc[:], dq[:])
                em = p2z.tile([128, 4], F32, tag="em", name=f"uem{b2}")
                nc.scalar.activation(em[:], mall[:], AF.Exp, scale=1.0 / OS)
                zt = p2z.tile([128, 4], F32, tag="zt", name=f"uzt{b2}")
                nc.vector.tensor_mul(zt[:], acc[:], em[:])
                zloc = agp.tile([128, 4], F32, tag="zloc", name=f"uzl{b2}")
                zglob = agp.tile([128, 4], F32, tag="zglob",
                                 name=f"uzg{b2}", addr_space="Shared")
                nc.sync.dma_start(zloc[:], zt[:])
                nc.gpsimd.collective_compute(
                    "AllReduce", mybir.AluOpType.add, replica_groups=RG,
                    ins=[zloc[:].opt()], outs=[zglob[:].opt()])
                zg_s = p2z.tile([128, 4], F32, tag="zgs", name=f"uzs{b2}")
                nc.sync.dma_start(zg_s[:], zglob[:])
                nc.scalar.activation(logZ[:, gsl], zg_s[:], AF.Ln)

            # remaining passB chunks
            for g in range(ngrp):
                for ci in range(4):
                    if (g, ci) in done_pb:
                        continue
                    v0 = VQ * ci
                    lin = p2pb.tile([128, VQ], BF16, tag="lin",
                                    name=f"uli{g}_{ci}")
                    nc.sync.dma_start(lin[:], logits_d[g, :, v0:v0 + VQ])
                    lout = p2pb.tile([128, VQ], BF16, tag="lout",
                                     name=f"ulo{g}_{ci}")
                    nc.vector.tensor_scalar(lout[:], lin[:], 1.0 / OS,
                                            logZ[:, g:g + 1],
                                            op0=mybir.AluOpType.mult,
                                            op1=mybir.AluOpType.subtract)
                    nc.sync.dma_start(
                        out_t[128 * g:128 * (g + 1), v0:v0 + VQ], lout[:])

    nc.compile()
    return nc


_NC_CACHE = {}


def _get_nc(ts):
    if ts not in _NC_CACHE:
        _NC_CACHE[ts] = build_nc(ts)
    return _NC_CACHE[ts]


def run_device(inputs, ts=TS_FULL, **run_kwargs):
    in_maps, ntok_pad = prep_inputs(inputs, ts)
    nc = _get_nc(ts)
    res = bass_utils.run_bass_kernel_spmd(nc, in_maps,
                                          core_ids=list(range(NC)), **run_kwargs)
    ntok = ts * B
    logp = np.empty((ntok, V), np.float32)
    for c in range(NC):
        logp[:, VS * c:VS * (c + 1)] = res.results[c]["out"][:ntok].astype(
            np.float32)
    out = np.zeros((B, T, V), np.float32)
    out[:, 0, SOS_ID] = 1.0
    out[:, 1:1 + ts, :] = logp.reshape(ts, B, V).transpose(1, 0, 2)
    return out, res


def kernel(**inputs) -> np.ndarray:
    out, _ = run_device(inputs, TS_FULL)
    return out


# revision 12
# speedup vs baseline: 1.0437x; 1.0437x over previous
"""Trainium2 Bass kernel for a 2-layer LSTM decoder + vocab projection + log-softmax.

v4 design notes (see git history for the baseline's layout, unchanged here):

- Recurrent matmuls are "flipped": stationary = h k-tile [128k x 32b],
  moving = W_hh (fp8, x2048) in 512-col streams, 4x column-tiled so four
  streams run concurrently on disjoint 32-col PE array groups. Each group
  accumulates in its own PSUM bank (HW requires one start=True chain per
  bank). Blocks are copied to SBUF and transposed back to the [128q, m*b]
  gate layout with small PE matmuls against a selection matrix S whose
  entries are 1/2048 - folding the fp8 weight scale away for free.

- Ticks are ordered so the PE queue never drains (HAM throttles the PE
  clock to 1.2 GHz after idle windows): projection matmul slices are
  interleaved between dependent recurrence segments, softmax stats run at
  tick end, and the layer-1 input pass borrows the flip-pool PSUM banks.

- W_out is fully resident as fp8 (x2048). All four vocab quarters of each
  token group, the z = sum(exp) AllReduce (replaces separate max+sum ARs),
  and the final logp = logits/2048 - logZ subtraction all run inside the
  recurrence; the tail handles only the last groups. Output is bf16.
"""

import numpy as np
import ml_dtypes
from contextlib import ExitStack

import concourse.bass as bass
import concourse.mybir as mybir
import concourse.tile as tile
from concourse import bacc
from concourse import bass_utils

F32 = mybir.dt.float32
BF16 = mybir.dt.bfloat16
FP8 = mybir.dt.float8e4
I32 = mybir.dt.int32
AF = mybir.ActivationFunctionType
bf16 = ml_dtypes.bfloat16
f8 = ml_dtypes.float8_e4m3fn

H = 1024
RH = 2048
V = 32000
B = 32
T = 128
NC = 8
GS = 1024          # gate rows per core per layer
HS = 256           # h dims per core
VS = V // NC       # vocab rows per core
VQ = VS // 4       # vocab quarter (1000)
D = 8              # layer-1 input-matmul batching (steps per weight pass)
LAG = D + 1        # layer-1 step lag behind layer 0
SOS_ID = 1
import os
OS = 2048.0        # fp8 weight scale
INREC_Z = os.environ.get("KV_INREC_Z", "0") == "1"
PROJ_INREC = os.environ.get("KV_PROJ_INREC", "1") == "1"
L1IN_FLP = os.environ.get("KV_L1IN_FLP", "1") == "1"
PROJ_SPLIT = os.environ.get("KV_PROJ_SPLIT", "0") == "1"

TS_FULL = T - 1    # 127 recurrence steps


def _p_major(w, kt, mt):
    """(kt*128, mt*128) -> (128, kt*mt*128) packed [p, k*mt*128 + m*128 + q]."""
    return np.ascontiguousarray(
        w.reshape(kt, 128, mt, 128).transpose(1, 0, 2, 3).reshape(128, kt * mt * 128)
    )


def _ktile_cols(a):
    """(kt*128, n) -> (128, kt*n) packed [p, k*n + j] = a[128k+p, j]."""
    kt = a.shape[0] // 128
    return np.ascontiguousarray(
        a.reshape(kt, 128, a.shape[1]).transpose(1, 0, 2).reshape(128, kt * a.shape[1])
    )


def prep_inputs(inp, ts=TS_FULL):
    """Host-side prep: slice/transpose/cast weights per core -> in_maps."""
    ntok_pad = ((ts * B + 127) // 128) * 128
    f32 = np.float32

    emb = np.asarray(inp["emb"], f32)
    tb = np.asarray(inp["target_batch"]).astype(np.int64)
    idx = tb[:, :ts].T.reshape(-1).astype(np.int32)       # (ts*B,) t-major
    idx = np.concatenate([idx, np.zeros(ntok_pad - idx.size, np.int32)])
    idx = np.ascontiguousarray(idx.reshape(ntok_pad // 128, 128).T)  # [p, group]

    ch = np.asarray(inp["context_h"], f32)
    cc = np.asarray(inp["context_c"], f32)
    h_init = np.concatenate([ch[0::2], ch[1::2]], axis=2)  # (2, B, RH)
    c_init = np.concatenate([cc[0::2], cc[1::2]], axis=2)

    def h_pack(hl):  # (B, RH) -> (128, 512) bf16 [p, 32k+b] = h[b, 128k+p]
        return np.ascontiguousarray(
            hl.T.reshape(16, 128, B).transpose(1, 0, 2).reshape(128, 16 * B)
        ).astype(bf16)

    Wih = [np.asarray(inp["W_ih0"], f32), np.asarray(inp["W_ih1"], f32)]
    Whh = [np.asarray(inp["W_hh0"], f32), np.asarray(inp["W_hh1"], f32)]
    bsum = [np.asarray(inp["b_ih0"], f32) + np.asarray(inp["b_hh0"], f32),
            np.asarray(inp["b_ih1"], f32) + np.asarray(inp["b_hh1"], f32)]
    W_out = np.asarray(inp["W_out"], f32)
    b_out = np.asarray(inp["b_out"], f32)

    # selection matrix with the 1/OS descale folded in
    S = np.zeros((128, B), np.float32)
    for g in range(4):
        S[32 * g + np.arange(B), np.arange(B)] = 1.0 / OS
    S = S.astype(bf16)

    in_maps = []
    for c in range(NC):
        # gate rows for core c, in i,f,o,g chunk order (256 rows each)
        rows = np.concatenate([np.arange(RH * k + HS * c, RH * k + HS * (c + 1))
                               for k in (0, 1, 3, 2)])  # i,f,o,g
        wih0t = _p_major(Wih[0][rows].T.astype(bf16), 8, 8)       # (128, 8192)
        whh0t = _p_major((Whh[0][rows].T * OS).astype(f8), 16, 8)  # fp8
        wih1t = _p_major((Wih[1][rows].T * OS).astype(f8), 16, 8)
        whh1t = _p_major((Whh[1][rows].T * OS).astype(f8), 16, 8)
        b0 = np.ascontiguousarray(bsum[0][rows].reshape(8, 128).T)  # (128, 8)
        b1 = np.ascontiguousarray(bsum[1][rows].reshape(8, 128).T)
        woutt = _ktile_cols(
            (W_out[VS * c:VS * (c + 1)].T * OS).astype(f8))   # (128, 64000) fp8
        boutc = (b_out[VS * c:VS * (c + 1)] * OS).reshape(1, VS).astype(bf16)

        def c_pack(cl):  # (B, RH) slice -> (128, 64) f32
            s = cl[:, HS * c:HS * (c + 1)].T  # (256, B)
            return np.ascontiguousarray(
                s.reshape(2, 128, B).transpose(1, 0, 2).reshape(128, 2 * B))

        in_maps.append({
            "idx": idx, "embt": emb, "smat": S,
            "wih0t": wih0t, "whh0t": whh0t, "wih1t": wih1t, "whh1t": whh1t,
            "b0": b0, "b1": b1, "woutt": woutt, "boutc": boutc,
            "h0init": h_pack(h_init[0]), "h1init": h_pack(h_init[1]),
            "c0init": c_pack(c_init[0]), "c1init": c_pack(c_init[1]),
        })
    return in_maps, ntok_pad


def build_nc(ts=TS_FULL):
    ntok_pad = ((ts * B + 127) // 128) * 128
    ntok = ts * B
    ngrp = ntok_pad // 128          # token groups of 128 for projection
    nticks = ts + LAG + 1

    nc = bacc.Bacc("TRN2", target_bir_lowering=False, debug=False,
                   enable_asserts=False, num_devices=NC)

    # ---- I/O ----
    idx_t = nc.dram_tensor("idx", [128, ntok_pad // 128], I32,
                           kind="ExternalInput").ap()
    emb_t = nc.dram_tensor("embt", [V, H], F32, kind="ExternalInput").ap()
    s_t = nc.dram_tensor("smat", [128, B], BF16, kind="ExternalInput").ap()
    wih0_t = nc.dram_tensor("wih0t", [128, 8 * GS], BF16, kind="ExternalInput").ap()
    whh0_t = nc.dram_tensor("whh0t", [128, 16 * GS], FP8, kind="ExternalInput").ap()
    wih1_t = nc.dram_tensor("wih1t", [128, 16 * GS], FP8, kind="ExternalInput").ap()
    whh1_t = nc.dram_tensor("whh1t", [128, 16 * GS], FP8, kind="ExternalInput").ap()
    b0_t = nc.dram_tensor("b0", [128, 8], F32, kind="ExternalInput").ap()
    b1_t = nc.dram_tensor("b1", [128, 8], F32, kind="ExternalInput").ap()
    wout_t = nc.dram_tensor("woutt", [128, 16 * VS], FP8, kind="ExternalInput").ap()
    bout_t = nc.dram_tensor("boutc", [1, VS], BF16, kind="ExternalInput").ap()
    h0i_t = nc.dram_tensor("h0init", [128, 512], BF16, kind="ExternalInput").ap()
    h1i_t = nc.dram_tensor("h1init", [128, 512], BF16, kind="ExternalInput").ap()
    c0i_t = nc.dram_tensor("c0init", [128, 64], F32, kind="ExternalInput").ap()
    c1i_t = nc.dram_tensor("c1init", [128, 64], F32, kind="ExternalInput").ap()
    out_t = nc.dram_tensor("out", [ntok_pad, VS], BF16, kind="ExternalOutput").ap()

    RG = [list(range(NC))]
    PCH = [(0, 512), (512, VQ - 512)]   # vocab chunk split per quarter

    with ExitStack() as ctx:
        tc = ctx.enter_context(tile.TileContext(nc))
        dram = ctx.enter_context(tc.tile_pool(name="dram", bufs=1, space="DRAM"))
        agp = ctx.enter_context(tc.tile_pool(name="agp", bufs=6, space="DRAM"))
        keep = ctx.enter_context(tc.tile_pool(name="keep", bufs=1))

        # long-lived: softmax stats, resident fp8 W_out, consts
        m4 = keep.tile([128, 4 * ngrp], F32, tag="m4")
        s4 = keep.tile([128, 4 * ngrp], F32, tag="s4")
        logZ = keep.tile([128, ngrp], F32, tag="logZ")
        ones_s = keep.tile([1, 128], BF16, tag="ones")
        bout_s = keep.tile([1, VS], BF16, tag="bouts")
        s_s = keep.tile([128, B], BF16, tag="ss")
        wout_s = keep.tile([128, 16 * VS], FP8, tag="wouts")

        # persistent DRAM
        xbf_d = dram.tile([ntok_pad, H], BF16, tag="xbf")
        g0_d = dram.tile([8, 128, ntok], BF16, tag="g0d")
        outs_d = dram.tile([ngrp, 16, 128, 128], BF16, tag="outsd")
        logits_d = dram.tile([ngrp, 128, VS], BF16, tag="logitsd")

        nc.gpsimd.memset(ones_s[:], 1.0)
        nc.sync.dma_start(bout_s[:], bout_t[:])
        nc.sync.dma_start(s_s[:], s_t[:])

        def proj_mms(g, q, osb, psq, kts, with_bias):
            """Projection matmul slice: k-tiles `kts` of quarter q."""
            v0 = VQ * q
            for k in kts:
                lhs = osb[:, 128 * k:128 * (k + 1)]
                for (o, w) in PCH:
                    nc.tensor.matmul(
                        psq[:, o:o + w], lhs,
                        wout_s[:, k * VS + v0 + o: k * VS + v0 + o + w],
                        start=(k == 0), stop=False)
            if with_bias:
                for (o, w) in PCH:
                    nc.tensor.matmul(psq[:, o:o + w], ones_s[:, :],
                                     bout_s[:, v0 + o:v0 + o + w],
                                     start=False, stop=True)

        def proj_stats(g, q, psq, scr_pool):
            gh = 4 * g + q
            v0 = VQ * q
            nc.vector.tensor_reduce(m4[:, gh:gh + 1], psq[:, :VQ],
                                    axis=mybir.AxisListType.X,
                                    op=mybir.AluOpType.max)
            negm = scr_pool.tile([128, 1], F32, tag="negm", name=f"nm{g}_{q}")
            nc.vector.tensor_scalar_mul(negm[:], m4[:, gh:gh + 1], -1.0 / OS)
            esc = scr_pool.tile([128, VQ], BF16, tag="esc", name=f"esc{g}_{q}")
            nc.scalar.activation(esc[:], psq[:, :VQ], AF.Exp,
                                 bias=negm[:, :1], scale=1.0 / OS,
                                 accum_out=s4[:, gh:gh + 1])
            lsb = scr_pool.tile([128, VQ], BF16, tag="lsb", name=f"lsb{g}_{q}")
            nc.vector.tensor_copy(lsb[:], psq[:, :VQ])
            nc.scalar.dma_start(logits_d[g, :, v0:v0 + VQ], lsb[:])

        # ============ Phase 0: embeddings + G0 = X @ Wih0.T + b0 ============
        with tc.tile_pool(name="rp", bufs=1) as rp:
            whh0_s = rp.tile([128, 16 * GS], FP8, tag="whh0s")
            b0_s = rp.tile([128, 8], F32, tag="b0s")
            b1_s = rp.tile([128, 8], F32, tag="b1s")
            h0ring = rp.tile([128, 16 * 512], BF16, tag="h0ring")   # 16 slots
            h1ring = rp.tile([128, 2 * 512], BF16, tag="h1ring")    # 2 slots
            g0ring = rp.tile([128, 2 * 2048], BF16, tag="g0ring")   # 2 x 8 steps
            g1ring = rp.tile([128, 2048], BF16, tag="g1ring")       # D steps [m,s,b]

            nc.sync.dma_start(whh0_s[:], whh0_t[:])
            nc.sync.dma_start(b0_s[:], b0_t[:])
            nc.sync.dma_start(b1_s[:], b1_t[:])
            # initial h into the ring slots read at t=0 / j=0
            nc.sync.dma_start(h0ring[:, 15 * 512:16 * 512], h0i_t[:])
            nc.sync.dma_start(h1ring[:, 1 * 512:2 * 512], h1i_t[:])

            TH = 2048  # token half for XT chunking
            with tc.tile_pool(name="p0sb", bufs=2) as p0sb, \
                 tc.tile_pool(name="p0ev", bufs=2) as p0ev, \
                 tc.tile_pool(name="p0big", bufs=1) as p0big, \
                 tc.tile_pool(name="p0ps", bufs=2, space="PSUM") as p0ps:
                idxs = p0big.tile([128, ntok_pad // 128], I32, tag="idxs")
                nc.sync.dma_start(idxs[:], idx_t[:])
                zpad = p0big.tile([128, 512], BF16, tag="zpad")
                nc.gpsimd.memset(zpad[:], 0.0)
                nc.scalar.dma_start(
                    outs_d[ngrp - 1, :, :, 96:128].rearrange("k p b -> p k b"),
                    zpad[:].rearrange("p (k b) -> p k b", k=16))
                for it in range(ntok_pad // 128):
                    xg = p0sb.tile([128, H], F32, tag="xg")
                    nc.gpsimd.indirect_dma_start(
                        out=xg[:], out_offset=None, in_=emb_t[:],
                        in_offset=bass.IndirectOffsetOnAxis(
                            ap=idxs[:, it:it + 1], axis=0))
                    xc = p0sb.tile([128, H], BF16, tag="xc")
                    nc.vector.tensor_copy(xc[:], xg[:])
                    nc.scalar.dma_start(xbf_d[128 * it:128 * (it + 1), :], xc[:])

                wih0_s = p0big.tile([128, 8 * GS], BF16, tag="wih0s")
                nc.sync.dma_start(wih0_s[:], wih0_t[:])
                xt_s = p0big.tile([128, 8 * TH], BF16, tag="xts")

                for half in range((ntok + TH - 1) // TH):
                    t0 = TH * half
                    tw = min(TH, ntok - t0)
                    twp = ((tw + 15) // 16) * 16  # transpose src rows mult of 16
                    for k in range(8):
                        nc.sync.dma_start_transpose(
                            xt_s[:, TH * k:TH * k + twp],
                            xbf_d[t0:t0 + twp, 128 * k:128 * (k + 1)])
                    nch = [(512 * i, min(512, tw - 512 * i))
                           for i in range((tw + 511) // 512)]
                    for m in range(8):
                        ps = p0ps.tile([128, 2048], F32, tag="p0ps")
                        for k in range(8):
                            lhs = wih0_s[:, k * GS + 128 * m: k * GS + 128 * (m + 1)]
                            for (o, w) in nch:
                                nc.tensor.matmul(
                                    ps[:, o:o + w], lhs,
                                    xt_s[:, TH * k + o: TH * k + o + w],
                                    start=(k == 0), stop=(k == 7))
                        ev = p0ev.tile([128, TH], BF16, tag="g0ev")
                        nc.scalar.activation(ev[:, :tw], ps[:, :tw], AF.Identity,
                                             bias=b0_s[:, m:m + 1])
                        nc.scalar.dma_start(g0_d[m, :, t0:t0 + tw], ev[:, :tw])

            # ============ Phase 1: recurrence ============
            c_prev = [None, None]
            done_quarters = set()
            done_zb = set()
            done_pb = set()
            with tc.tile_pool(name="rp2", bufs=1) as rp2, \
                 tc.tile_pool(name="flp", bufs=1, space="PSUM") as flp, \
                 tc.tile_pool(name="gps", bufs=1, space="PSUM") as gps, \
                 tc.tile_pool(name="shp", bufs=1, space="PSUM") as shp, \
                 tc.tile_pool(name="gsbp", bufs=2) as gsbp, \
                 tc.tile_pool(name="posb", bufs=1) as posb_pool, \
                 tc.tile_pool(name="pscr", bufs=1) as pscr_pool, \
                 tc.tile_pool(name="pbp", bufs=2) as pbp, \
                 tc.tile_pool(name="zp", bufs=1) as zp, \
                 tc.tile_pool(name="cell", bufs=2) as cell_pool:

                wih1_s = rp2.tile([128, 16 * GS], FP8, tag="wih1s")
                whh1_s = rp2.tile([128, 16 * GS], FP8, tag="whh1s")
                nc.scalar.dma_start(wih1_s[:], wih1_t[:])
                nc.scalar.dma_start(whh1_s[:], whh1_t[:])
                nc.scalar.dma_start(wout_s[:], wout_t[:])

                def g0_prefetch(blk):
                    t0 = 8 * blk
                    nsteps = min(8, ts - t0)
                    if nsteps <= 0:
                        return
                    dst = g0ring[:].rearrange("p (h m s b) -> p h m s b",
                                              h=2, m=8, b=B)
                    src = g0_d[:, :, B * t0: B * (t0 + nsteps)].rearrange(
                        "m p sb -> p m sb")
                    nc.scalar.dma_start(
                        dst[:, blk % 2, :, 0:nsteps, :].rearrange(
                            "p m s b -> p m (s b)"), src)

                def flip_mms(w_s, h_of_kt, li):
                    """Flipped hh matmul into 4 one-bank psum tiles."""
                    Ps = [flp.tile([128, 512], F32, tag=f"fl{g}",
                                   name=f"fl{g}_{li}") for g in range(4)]
                    for r in range(8):
                        for g in range(4):
                            jm, jk = g >> 1, g & 1
                            kt = 8 * jk + r
                            nc.tensor.matmul(
                                Ps[g][32 * g:32 * (g + 1), :],
                                h_of_kt(kt),
                                w_s[:, kt * GS + 512 * jm: kt * GS + 512 * jm + 512],
                                start=(r == 0), stop=(r == 7),
                                tile_position=(0, 32 * g))
                    gsb = gsbp.tile([128, 1024], BF16, tag="gsb",
                                    name=f"gsb{li}")
                    for g in range(4):
                        jm = g >> 1
                        dst = gsb[32 * g:32 * (g + 1), 512 * jm:512 * jm + 512]
                        src = Ps[g][32 * g:32 * (g + 1), :]
                        if g % 2 == 0:
                            nc.scalar.activation(dst, src, AF.Identity)
                        else:
                            nc.vector.tensor_copy(dst, src)
                    return gsb

                def combine(gsb, li):
                    """Transpose-combine gsb blocks -> G [128q, 8m x 32b]."""
                    G = gps.tile([128, 512], F32, tag=f"G{li}", name=f"G{li}")
                    for cch in range(8):
                        jm = cch // 4
                        nc.tensor.matmul(
                            G[:, B * cch:B * (cch + 1)],
                            gsb[64 * jm:64 * jm + 64, 128 * cch:128 * (cch + 1)],
                            s_s[64 * jm:64 * jm + 64, :],
                            start=True, stop=True)
                    return G

                def cell(l, G, gadd_ap):
                    """LSTM cell for layer l; returns hn tile [128, 64] bf16."""
                    g = cell_pool.tile([128, 256], F32, tag=f"g{l}")
                    nc.vector.tensor_add(
                        g[:].rearrange("p (m b) -> p m b", b=B),
                        G[:, :256].rearrange("p (m b) -> p m b", b=B),
                        gadd_ap)
                    sfo = cell_pool.tile([128, 192], F32, tag=f"sfo{l}")
                    nc.scalar.activation(sfo[:], g[:, 0:192], AF.Sigmoid)
                    tg = cell_pool.tile([128, 64], F32, tag=f"tg{l}")
                    nc.scalar.activation(tg[:], g[:, 192:256], AF.Tanh)
                    t1 = cell_pool.tile([128, 64], F32, tag=f"t1{l}")
                    nc.vector.tensor_mul(t1[:], sfo[:, 0:64], tg[:])
                    t2 = cell_pool.tile([128, 64], F32, tag=f"t2{l}")
                    nc.vector.tensor_mul(t2[:], sfo[:, 64:128], c_prev[l][:])
                    cn = cell_pool.tile([128, 64], F32, tag=f"cn{l}")
                    nc.vector.tensor_add(cn[:], t1[:], t2[:])
                    c_prev[l] = cn
                    tcn = cell_pool.tile([128, 64], F32, tag=f"tc{l}")
                    nc.scalar.activation(tcn[:], cn[:], AF.Tanh)
                    hn = cell_pool.tile([128, 64], BF16, tag=f"hn{l}")
                    nc.vector.tensor_mul(hn[:], sfo[:, 128:192], tcn[:])
                    return hn

                def zbatch(b):
                    """z-AllReduce + logZ for groups 4b..4b+3."""
                    gsl = slice(4 * b, 4 * b + 4)
                    m4v = m4[:].rearrange("p (g q) -> p g q", q=4)[:, gsl, :]
                    s4v = s4[:].rearrange("p (g q) -> p g q", q=4)[:, gsl, :]
                    t01 = zp.tile([128, 4], F32, tag="t01", name=f"t01_{b}")
                    t23 = zp.tile([128, 4], F32, tag="t23", name=f"t23_{b}")
                    nc.vector.tensor_max(t01[:], m4v[:, :, 0], m4v[:, :, 1])
                    nc.vector.tensor_max(t23[:], m4v[:, :, 2], m4v[:, :, 3])
                    mall = zp.tile([128, 4], F32, tag="mall", name=f"ma{b}")
                    nc.vector.tensor_max(mall[:], t01[:], t23[:])
                    acc = zp.tile([128, 4], F32, tag="acc", name=f"ac{b}")
                    dq = zp.tile([128, 4], F32, tag="dq", name=f"dq{b}")
                    for q in range(4):
                        nc.vector.tensor_sub(dq[:], m4v[:, :, q], mall[:])
                        nc.scalar.activation(dq[:], dq[:], AF.Exp, scale=1.0 / OS)
                        nc.vector.tensor_mul(dq[:], dq[:], s4v[:, :, q])
                        if q == 0:
                            nc.vector.tensor_copy(acc[:], dq[:])
                        else:
                            nc.vector.tensor_add(acc[:], acc[:], dq[:])
                    em = zp.tile([128, 4], F32, tag="em", name=f"em{b}")
                    nc.scalar.activation(em[:], mall[:], AF.Exp, scale=1.0 / OS)
                    zt = zp.tile([128, 4], F32, tag="zt", name=f"zt{b}")
                    nc.vector.tensor_mul(zt[:], acc[:], em[:])
                    zloc = agp.tile([128, 4], F32, tag="zloc", name=f"zl{b}")
                    zglob = agp.tile([128, 4], F32, tag="zglob",
                                     name=f"zg{b}", addr_space="Shared")
                    nc.sync.dma_start(zloc[:], zt[:])
                    nc.gpsimd.collective_compute(
                        "AllReduce", mybir.AluOpType.add, replica_groups=RG,
                        ins=[zloc[:].opt()], outs=[zglob[:].opt()])
                    zg_s = zp.tile([128, 4], F32, tag="zgs", name=f"zs{b}")
                    nc.sync.dma_start(zg_s[:], zglob[:])
                    nc.scalar.activation(logZ[:, gsl], zg_s[:], AF.Ln)
                    done_zb.add(b)

                def passb_chunk(g, ci):
                    """logp chunk: out[128g.., 1000ci..] = lin/OS - logZ[g]."""
                    v0 = VQ * ci
                    lin = pbp.tile([128, VQ], BF16, tag="lin",
                                   name=f"li{g}_{ci}")
                    nc.sync.dma_start(lin[:], logits_d[g, :, v0:v0 + VQ])
                    lout = pbp.tile([128, VQ], BF16, tag="lout",
                                    name=f"lo{g}_{ci}")
                    nc.vector.tensor_scalar(lout[:], lin[:], 1.0 / OS,
                                            logZ[:, g:g + 1],
                                            op0=mybir.AluOpType.mult,
                                            op1=mybir.AluOpType.subtract)
                    nc.sync.dma_start(
                        out_t[128 * g:128 * (g + 1), v0:v0 + VQ], lout[:])
                    done_pb.add((g, ci))

                c0s = cell_pool.tile([128, 64], F32, tag="cn0")
                nc.sync.dma_start(c0s[:], c0i_t[:])
                c_prev[0] = c0s
                c1s = cell_pool.tile([128, 64], F32, tag="cn1")
                nc.sync.dma_start(c1s[:], c1i_t[:])
                c_prev[1] = c1s

                g0_prefetch(0)
                g0_prefetch(1)

                h0r4 = h0ring[:].rearrange("p (s k b) -> p s k b", s=16, b=B)
                h1r4 = h1ring[:].rearrange("p (s k b) -> p s k b", s=2, b=B)
                g0r5 = g0ring[:].rearrange("p (h m s b) -> p h m s b",
                                           h=2, m=8, b=B)
                g1r4 = g1ring[:].rearrange("p (m s b) -> p m s b", m=8, b=B)

                # schedules: one projection quarter per tick; z-AR per 4
                # groups; one passB chunk per tick after logZ is known
                psched = {}
                if PROJ_INREC:
                    for g in range(ngrp - 1):
                        for q in range(4):
                            t_q = 4 * g + 14 + q
                            if t_q < nticks:
                                psched[t_q] = (g, q)
                zsched = {}
                pbsched = {}
                if INREC_Z:
                    for b2 in range((ngrp + 3) // 4):
                        tz = 16 * b2 + 31
                        if tz < nticks:
                            zsched[tz] = b2
                        for i in range(16):
                            tp = 16 * b2 + 32 + i
                            if tp < nticks:
                                pbsched[tp] = (4 * b2 + i // 4, i % 4)
                cur_osb = [None]

                # split per-layer AllGathers: each is triggered right after
                # its cell and consumed ~a full tick later, so the ~10us
                # collective+DMA chain stays off the critical path.
                for t in range(nticks):
                    j = t - LAG  # layer-1 step this tick

                    pj = psched.get(t)
                    psq = None
                    if pj is not None:
                        g, q = pj

                    # ---- layer 0, step t ----
                    if t < ts:
                        gsb0 = flip_mms(
                            whh0_s,
                            lambda kt: h0r4[:, (t - 1) % 16, kt, :], "a")
                        G0 = combine(gsb0, "a")
                        if t % 8 == 7:
                            g0_prefetch(t // 8 + 2)
                        hn0 = cell(0, G0, g0r5[:, (t // 8) % 2, :, t % 8, :])
                        agin0 = agp.tile([2, 128, B], BF16, tag="agin0",
                                         name=f"ai0_{t}")
                        nc.sync.dma_start(
                            agin0[:].rearrange("j p b -> p j b"),
                            hn0[:].rearrange("p (j b) -> p j b", b=B))
                        agout0 = agp.tile([NC, 2, 128, B], BF16, tag="agout0",
                                          name=f"ao0_{t}", addr_space="Shared")
                        nc.gpsimd.collective_compute(
                            "AllGather", mybir.AluOpType.bypass,
                            replica_groups=RG,
                            ins=[agin0[:].opt()], outs=[agout0[:].opt()])
                        nc.sync.dma_start(
                            h0r4[:, t % 16, 0:8, :],
                            agout0[0:4].rearrange("r j p b -> p (r j) b"))
                        nc.gpsimd.dma_start(
                            h0r4[:, t % 16, 8:16, :],
                            agout0[4:8].rearrange("r j p b -> p (r j) b"))

                    # ---- projection quarter (fills the AG flight window) ----
                    if pj is not None:
                        if q == 0:
                            osb = posb_pool.tile([128, 2048], BF16, tag="posb",
                                                 name=f"osb{g}")
                            nc.sync.dma_start(
                                osb[:].rearrange("p (k q) -> p k q", k=16),
                                outs_d[g, :, :, :].rearrange("k p q -> p k q"))
                            cur_osb[0] = osb
                        psq = shp.tile([128, 1024], F32, tag="sh",
                                       name=f"psq{g}_{q}")
                        proj_mms(g, q, cur_osb[0], psq, range(16), True)

                    # ---- layer-1 input pass every D steps ----
                    if 0 <= j < ts and j % D == 0:
                        nb = min(D, ts - j)
                        s0 = j % 16
                        for qi in range(4):
                            psg = flp.tile([128, 512], F32, tag=f"fl{qi}",
                                           name=f"psg{t}_{qi}")
                            for lmi in range(2):
                                m = 2 * qi + lmi
                                for k in range(16):
                                    nc.tensor.matmul(
                                        psg[:, 256 * lmi: 256 * lmi + B * nb],
                                        wih1_s[:, k * GS + 128 * m:
                                               k * GS + 128 * (m + 1)],
                                        h0r4[:, s0:s0 + nb, k, :],
                                        start=(k == 0), stop=(k == 15))
                            for lmi in range(2):
                                m = 2 * qi + lmi
                                nc.scalar.activation(
                                    g1ring[:, 256 * m: 256 * m + B * nb],
                                    psg[:, 256 * lmi: 256 * lmi + B * nb],
                                    AF.Identity, scale=1.0 / OS,
                                    bias=b1_s[:, m:m + 1])

                    # ---- layer 1, step j ----
                    if 0 <= j < ts:
                        gsb1 = flip_mms(
                            whh1_s,
                            lambda kt: h1r4[:, (j - 1) % 2, kt, :], "b")
                        G1 = combine(gsb1, "b")
                        hn1 = cell(1, G1, g1r4[:, :, j % D, :])
                        agin1 = agp.tile([2, 128, B], BF16, tag="agin1",
                                         name=f"ai1_{t}")
                        nc.sync.dma_start(
                            agin1[:].rearrange("j p b -> p j b"),
                            hn1[:].rearrange("p (j b) -> p j b", b=B))
                        agout1 = agp.tile([NC, 2, 128, B], BF16, tag="agout1",
                                          name=f"ao1_{t}", addr_space="Shared")
                        nc.gpsimd.collective_compute(
                            "AllGather", mybir.AluOpType.bypass,
                            replica_groups=RG,
                            ins=[agin1[:].opt()], outs=[agout1[:].opt()])
                        nc.sync.dma_start(
                            h1r4[:, j % 2, 0:8, :],
                            agout1[0:4].rearrange("r j p b -> p (r j) b"))
                        nc.gpsimd.dma_start(
                            h1r4[:, j % 2, 8:16, :],
                            agout1[4:8].rearrange("r j p b -> p (r j) b"))
                        nc.scalar.dma_start(
                            outs_d[j // 4, :, :, B * (j % 4):B * (j % 4 + 1)]
                            .rearrange("k p b -> p k b"),
                            h1r4[:, j % 2, :, :])

                    # stats at tick end (keeps DVE/ACT off the PE's path)
                    if pj is not None:
                        proj_stats(g, q, psq, pscr_pool)
                        done_quarters.add((g, q))
                    if t in zsched:
                        zbatch(zsched[t])
                    if t in pbsched:
                        gpb, ci = pbsched[t]
                        passb_chunk(gpb, ci)

        # ============ Phase 2 tail: leftovers ============
        with tc.tile_pool(name="p2sb", bufs=2) as p2sb, \
             tc.tile_pool(name="p2scr", bufs=2) as p2scr, \
             tc.tile_pool(name="p2z", bufs=1) as p2z, \
             tc.tile_pool(name="p2pb", bufs=2) as p2pb, \
             tc.tile_pool(name="p2ps", bufs=2, space="PSUM") as p2ps:

            # remaining projection quarters
            for g in range(ngrp):
                rem = [q for q in range(4) if (g, q) not in done_quarters]
                if not rem:
                    continue
                osb = p2sb.tile([128, 2048], BF16, tag="osb", name=f"osbt{g}")
                nc.sync.dma_start(
                    osb[:].rearrange("p (k q) -> p k q", k=16),
                    outs_d[g, :, :, :].rearrange("k p q -> p k q"))
                for q in rem:
                    psq = p2ps.tile([128, 1024], F32, tag="sh",
                                    name=f"tps{g}_{q}")
                    proj_mms(g, q, osb, psq, range(16), True)
                    proj_stats(g, q, psq, p2scr)

            # remaining z batches (reuse the in-rec helper pools shapes)
            nb2 = (ngrp + 3) // 4
            for b2 in range(nb2):
                if b2 in done_zb:
                    continue
                gsl = slice(4 * b2, 4 * b2 + 4)
                m4v = m4[:].rearrange("p (g q) -> p g q", q=4)[:, gsl, :]
                s4v = s4[:].rearrange("p (g q) -> p g q", q=4)[:, gsl, :]
                t01 = p2z.tile([128, 4], F32, tag="t01", name=f"u01_{b2}")
                t23 = p2z.tile([128, 4], F32, tag="t23", name=f"u23_{b2}")
                nc.vector.tensor_max(t01[:], m4v[:, :, 0], m4v[:, :, 1])
                nc.vector.tensor_max(t23[:], m4v[:, :, 2], m4v[:, :, 3])
                mall = p2z.tile([128, 4], F32, tag="mall", name=f"uma{b2}")
                nc.vector.tensor_max(mall[:], t01[:], t23[:])
                acc = p2z.tile([128, 4], F32, tag="acc", name=f"uac{b2}")
                dq = p2z.tile([128, 4], F32, tag="dq", name=f"udq{b2}")
                for q in range(4):
                    nc.vector.tensor_sub(dq[:], m4v[:, :, q], mall[:])
                    nc.scalar.activation(dq[:], dq[:], AF.Exp, scale=1.0 / OS)
                    nc.vector.tensor_mul(dq[:], dq[:], s4v[:, :, q])
                    if q == 0:
                        nc.vector.tensor_copy(acc[:], dq[:])
                    else:
                        nc.vector.tensor_add(acc[:], acc[:], dq[:])
                em = p2z.tile([128, 4], F32, tag="em", name=f"uem{b2}")
                nc.scalar.activation(em[:], mall[:], AF.Exp, scale=1.0 / OS)
                zt = p2z.tile([128, 4], F32, tag="zt", name=f"uzt{b2}")
                nc.vector.tensor_mul(zt[:], acc[:], em[:])
                zloc = agp.tile([128, 4], F32, tag="zloc", name=f"uzl{b2}")
                zglob = agp.tile([128, 4], F32, tag="zglob",
                                 name=f"uzg{b2}", addr_space="Shared")
                nc.sync.dma_start(zloc[:], zt[:])
                nc.gpsimd.collective_compute(
                    "AllReduce", mybir.AluOpType.add, replica_groups=RG,
                    ins=[zloc[:].opt()], outs=[zglob[:].opt()])
                zg_s = p2z.tile([128, 4], F32, tag="zgs", name=f"uzs{b2}")
                nc.sync.dma_start(zg_s[:], zglob[:])
                nc.scalar.activation(logZ[:, gsl], zg_s[:], AF.Ln)

            # remaining passB chunks
            for g in range(ngrp):
                for ci in range(4):
                    if (g, ci) in done_pb:
                        continue
                    v0 = VQ * ci
                    lin = p2pb.tile([128, VQ], BF16, tag="lin",
                                    name=f"uli{g}_{ci}")
                    nc.sync.dma_start(lin[:], logits_d[g, :, v0:v0 + VQ])
                    lout = p2pb.tile([128, VQ], BF16, tag="lout",
                                     name=f"ulo{g}_{ci}")
                    nc.vector.tensor_scalar(lout[:], lin[:], 1.0 / OS,
                                            logZ[:, g:g + 1],
                                            op0=mybir.AluOpType.mult,
                                            op1=mybir.AluOpType.subtract)
                    nc.sync.dma_start(
                        out_t[128 * g:128 * (g + 1), v0:v0 + VQ], lout[:])

    nc.compile()
    return nc


_NC_CACHE = {}


def _get_nc(ts):
    if ts not in _NC_CACHE:
        _NC_CACHE[ts] = build_nc(ts)
    return _NC_CACHE[ts]


def run_device(inputs, ts=TS_FULL, **run_kwargs):
    in_maps, ntok_pad = prep_inputs(inputs, ts)
    nc = _get_nc(ts)
    res = bass_utils.run_bass_kernel_spmd(nc, in_maps,
                                          core_ids=list(range(NC)), **run_kwargs)
    ntok = ts * B
    logp = np.empty((ntok, V), np.float32)
    for c in range(NC):
        logp[:, VS * c:VS * (c + 1)] = res.results[c]["out"][:ntok].astype(
            np.float32)
    out = np.zeros((B, T, V), np.float32)
    out[:, 0, SOS_ID] = 1.0
    out[:, 1:1 + ts, :] = logp.reshape(ts, B, V).transpose(1, 0, 2)
    return out, res


def kernel(**inputs) -> np.ndarray:
    out, _ = run_device(inputs, TS_FULL)
    return out


# revision 13
# speedup vs baseline: 1.1210x; 1.0740x over previous
"""Trainium2 Bass kernel for a 2-layer LSTM decoder + vocab projection + log-softmax.

v4 design notes (see git history for the baseline's layout, unchanged here):

- Recurrent matmuls are "flipped": stationary = h k-tile [128k x 32b],
  moving = W_hh (fp8, x2048) in 512-col streams, 4x column-tiled so four
  streams run concurrently on disjoint 32-col PE array groups. Each group
  accumulates in its own PSUM bank (HW requires one start=True chain per
  bank). Blocks are copied to SBUF and transposed back to the [128q, m*b]
  gate layout with small PE matmuls against a selection matrix S whose
  entries are 1/2048 - folding the fp8 weight scale away for free.

- Ticks are ordered so the PE queue never drains (HAM throttles the PE
  clock to 1.2 GHz after idle windows): projection matmul slices are
  interleaved between dependent recurrence segments, softmax stats run at
  tick end, and the layer-1 input pass borrows the flip-pool PSUM banks.

- W_out is fully resident as fp8 (x2048). All four vocab quarters of each
  token group, the z = sum(exp) AllReduce (replaces separate max+sum ARs),
  and the final logp = logits/2048 - logZ subtraction all run inside the
  recurrence; the tail handles only the last groups. Output is bf16.
"""

import numpy as np
import ml_dtypes
from contextlib import ExitStack

import concourse.bass as bass
import concourse.mybir as mybir
import concourse.tile as tile
from concourse import bacc
from concourse import bass_utils

F32 = mybir.dt.float32
BF16 = mybir.dt.bfloat16
FP8 = mybir.dt.float8e4
I32 = mybir.dt.int32
AF = mybir.ActivationFunctionType
bf16 = ml_dtypes.bfloat16
f8 = ml_dtypes.float8_e4m3fn

H = 1024
RH = 2048
V = 32000
B = 32
T = 128
NC = 8
GS = 1024          # gate rows per core per layer
HS = 256           # h dims per core
VS = V // NC       # vocab rows per core
VQ = VS // 4       # vocab quarter (1000)
D = 8              # layer-1 input-matmul batching (steps per weight pass)
LAG = D + 1        # layer-1 step lag behind layer 0
SOS_ID = 1
import os
OS = 2048.0        # fp8 weight scale
INREC_Z = os.environ.get("KV_INREC_Z", "0") == "1"
PROJ_INREC = os.environ.get("KV_PROJ_INREC", "1") == "1"
L1IN_FLP = os.environ.get("KV_L1IN_FLP", "1") == "1"
PROJ_SPLIT = os.environ.get("KV_PROJ_SPLIT", "0") == "1"

TS_FULL = T - 1    # 127 recurrence steps


def _p_major(w, kt, mt):
    """(kt*128, mt*128) -> (128, kt*mt*128) packed [p, k*mt*128 + m*128 + q]."""
    return np.ascontiguousarray(
        w.reshape(kt, 128, mt, 128).transpose(1, 0, 2, 3).reshape(128, kt * mt * 128)
    )


def _ktile_cols(a):
    """(kt*128, n) -> (128, kt*n) packed [p, k*n + j] = a[128k+p, j]."""
    kt = a.shape[0] // 128
    return np.ascontiguousarray(
        a.reshape(kt, 128, a.shape[1]).transpose(1, 0, 2).reshape(128, kt * a.shape[1])
    )


def prep_inputs(inp, ts=TS_FULL):
    """Host-side prep: slice/transpose/cast weights per core -> in_maps."""
    ntok_pad = ((ts * B + 127) // 128) * 128
    f32 = np.float32

    emb = np.asarray(inp["emb"], f32)
    tb = np.asarray(inp["target_batch"]).astype(np.int64)
    idx = tb[:, :ts].T.reshape(-1).astype(np.int32)       # (ts*B,) t-major
    idx = np.concatenate([idx, np.zeros(ntok_pad - idx.size, np.int32)])
    idx = np.ascontiguousarray(idx.reshape(ntok_pad // 128, 128).T)  # [p, group]

    ch = np.asarray(inp["context_h"], f32)
    cc = np.asarray(inp["context_c"], f32)
    h_init = np.concatenate([ch[0::2], ch[1::2]], axis=2)  # (2, B, RH)
    c_init = np.concatenate([cc[0::2], cc[1::2]], axis=2)

    def h_pack(hl):  # (B, RH) -> (128, 512) bf16 [p, 32k+b] = h[b, 128k+p]
        return np.ascontiguousarray(
            hl.T.reshape(16, 128, B).transpose(1, 0, 2).reshape(128, 16 * B)
        ).astype(bf16)

    Wih = [np.asarray(inp["W_ih0"], f32), np.asarray(inp["W_ih1"], f32)]
    Whh = [np.asarray(inp["W_hh0"], f32), np.asarray(inp["W_hh1"], f32)]
    bsum = [np.asarray(inp["b_ih0"], f32) + np.asarray(inp["b_hh0"], f32),
            np.asarray(inp["b_ih1"], f32) + np.asarray(inp["b_hh1"], f32)]
    W_out = np.asarray(inp["W_out"], f32)
    b_out = np.asarray(inp["b_out"], f32)

    # selection matrix with the 1/OS descale folded in
    S = np.zeros((128, B), np.float32)
    for g in range(4):
        S[32 * g + np.arange(B), np.arange(B)] = 1.0 / OS
    S = S.astype(bf16)

    in_maps = []
    for c in range(NC):
        # gate rows for core c, in i,f,o,g chunk order (256 rows each)
        rows = np.concatenate([np.arange(RH * k + HS * c, RH * k + HS * (c + 1))
                               for k in (0, 1, 3, 2)])  # i,f,o,g
        wih0t = _p_major(Wih[0][rows].T.astype(bf16), 8, 8)       # (128, 8192)
        whh0t = _p_major((Whh[0][rows].T * OS).astype(f8), 16, 8)  # fp8
        wih1t = _p_major((Wih[1][rows].T * OS).astype(f8), 16, 8)
        whh1t = _p_major((Whh[1][rows].T * OS).astype(f8), 16, 8)
        b0 = np.ascontiguousarray(bsum[0][rows].reshape(8, 128).T)  # (128, 8)
        b1 = np.ascontiguousarray(bsum[1][rows].reshape(8, 128).T)
        woutt = _ktile_cols(
            (W_out[VS * c:VS * (c + 1)].T * OS).astype(f8))   # (128, 64000) fp8
        boutc = (b_out[VS * c:VS * (c + 1)] * OS).reshape(1, VS).astype(bf16)

        def c_pack(cl):  # (B, RH) slice -> (128, 64) f32
            s = cl[:, HS * c:HS * (c + 1)].T  # (256, B)
            return np.ascontiguousarray(
                s.reshape(2, 128, B).transpose(1, 0, 2).reshape(128, 2 * B))

        in_maps.append({
            "idx": idx, "embt": emb, "smat": S,
            "wih0t": wih0t, "whh0t": whh0t, "wih1t": wih1t, "whh1t": whh1t,
            "b0": b0, "b1": b1, "woutt": woutt, "boutc": boutc,
            "h0init": h_pack(h_init[0]), "h1init": h_pack(h_init[1]),
            "c0init": c_pack(c_init[0]), "c1init": c_pack(c_init[1]),
        })
    return in_maps, ntok_pad


def build_nc(ts=TS_FULL):
    ntok_pad = ((ts * B + 127) // 128) * 128
    ntok = ts * B
    ngrp = ntok_pad // 128          # token groups of 128 for projection
    nticks = ts + LAG + 1

    nc = bacc.Bacc("TRN2", target_bir_lowering=False, debug=False,
                   enable_asserts=False, num_devices=NC)

    # ---- I/O ----
    idx_t = nc.dram_tensor("idx", [128, ntok_pad // 128], I32,
                           kind="ExternalInput").ap()
    emb_t = nc.dram_tensor("embt", [V, H], F32, kind="ExternalInput").ap()
    s_t = nc.dram_tensor("smat", [128, B], BF16, kind="ExternalInput").ap()
    wih0_t = nc.dram_tensor("wih0t", [128, 8 * GS], BF16, kind="ExternalInput").ap()
    whh0_t = nc.dram_tensor("whh0t", [128, 16 * GS], FP8, kind="ExternalInput").ap()
    wih1_t = nc.dram_tensor("wih1t", [128, 16 * GS], FP8, kind="ExternalInput").ap()
    whh1_t = nc.dram_tensor("whh1t", [128, 16 * GS], FP8, kind="ExternalInput").ap()
    b0_t = nc.dram_tensor("b0", [128, 8], F32, kind="ExternalInput").ap()
    b1_t = nc.dram_tensor("b1", [128, 8], F32, kind="ExternalInput").ap()
    wout_t = nc.dram_tensor("woutt", [128, 16 * VS], FP8, kind="ExternalInput").ap()
    bout_t = nc.dram_tensor("boutc", [1, VS], BF16, kind="ExternalInput").ap()
    h0i_t = nc.dram_tensor("h0init", [128, 512], BF16, kind="ExternalInput").ap()
    h1i_t = nc.dram_tensor("h1init", [128, 512], BF16, kind="ExternalInput").ap()
    c0i_t = nc.dram_tensor("c0init", [128, 64], F32, kind="ExternalInput").ap()
    c1i_t = nc.dram_tensor("c1init", [128, 64], F32, kind="ExternalInput").ap()
    out_t = nc.dram_tensor("out", [ntok_pad, VS], BF16, kind="ExternalOutput").ap()

    RG = [list(range(NC))]
    PCH = [(0, 512), (512, VQ - 512)]   # vocab chunk split per quarter

    with ExitStack() as ctx:
        tc = ctx.enter_context(tile.TileContext(nc))
        dram = ctx.enter_context(tc.tile_pool(name="dram", bufs=1, space="DRAM"))
        agp = ctx.enter_context(tc.tile_pool(name="agp", bufs=6, space="DRAM"))
        keep = ctx.enter_context(tc.tile_pool(name="keep", bufs=1))

        # long-lived: softmax stats, resident fp8 W_out, consts
        m4 = keep.tile([128, 4 * ngrp], F32, tag="m4")
        s4 = keep.tile([128, 4 * ngrp], F32, tag="s4")
        logZ = keep.tile([128, ngrp], F32, tag="logZ")
        ones_s = keep.tile([1, 128], BF16, tag="ones")
        bout_s = keep.tile([1, VS], BF16, tag="bouts")
        s_s = keep.tile([128, B], BF16, tag="ss")
        wout_s = keep.tile([128, 16 * VS], FP8, tag="wouts")

        # persistent DRAM
        xbf_d = dram.tile([ntok_pad, H], BF16, tag="xbf")
        g0_d = dram.tile([8, 128, ntok], BF16, tag="g0d")
        outs_d = dram.tile([ngrp, 16, 128, 128], BF16, tag="outsd")
        logits_d = dram.tile([ngrp, 128, VS], BF16, tag="logitsd")

        nc.gpsimd.memset(ones_s[:], 1.0)
        nc.sync.dma_start(bout_s[:], bout_t[:])
        nc.sync.dma_start(s_s[:], s_t[:])

        def proj_mms(g, q, osb, psq, kts, with_bias):
            """Projection matmul slice: k-tiles `kts` of quarter q."""
            v0 = VQ * q
            for k in kts:
                lhs = osb[:, 128 * k:128 * (k + 1)]
                for (o, w) in PCH:
                    nc.tensor.matmul(
                        psq[:, o:o + w], lhs,
                        wout_s[:, k * VS + v0 + o: k * VS + v0 + o + w],
                        start=(k == 0), stop=False)
            if with_bias:
                for (o, w) in PCH:
                    nc.tensor.matmul(psq[:, o:o + w], ones_s[:, :],
                                     bout_s[:, v0 + o:v0 + o + w],
                                     start=False, stop=True)

        def proj_stats(g, q, psq, scr_pool):
            gh = 4 * g + q
            v0 = VQ * q
            nc.vector.tensor_reduce(m4[:, gh:gh + 1], psq[:, :VQ],
                                    axis=mybir.AxisListType.X,
                                    op=mybir.AluOpType.max)
            negm = scr_pool.tile([128, 1], F32, tag="negm", name=f"nm{g}_{q}")
            nc.vector.tensor_scalar_mul(negm[:], m4[:, gh:gh + 1], -1.0 / OS)
            esc = scr_pool.tile([128, VQ], BF16, tag="esc", name=f"esc{g}_{q}")
            nc.scalar.activation(esc[:], psq[:, :VQ], AF.Exp,
                                 bias=negm[:, :1], scale=1.0 / OS,
                                 accum_out=s4[:, gh:gh + 1])
            lsb = scr_pool.tile([128, VQ], BF16, tag="lsb", name=f"lsb{g}_{q}")
            nc.vector.tensor_copy(lsb[:], psq[:, :VQ])
            nc.scalar.dma_start(logits_d[g, :, v0:v0 + VQ], lsb[:])

        # ============ Phase 0: embeddings + G0 = X @ Wih0.T + b0 ============
        with tc.tile_pool(name="rp", bufs=1) as rp:
            whh0_s = rp.tile([128, 16 * GS], FP8, tag="whh0s")
            b0_s = rp.tile([128, 8], F32, tag="b0s")
            b1_s = rp.tile([128, 8], F32, tag="b1s")
            h0ring = rp.tile([128, 16 * 512], BF16, tag="h0ring")   # 16 slots
            h1ring = rp.tile([128, 4 * 512], BF16, tag="h1ring")    # 4 slots
            g0ring = rp.tile([128, 2 * 2048], BF16, tag="g0ring")   # 2 x 8 steps
            g1ring = rp.tile([128, 2048], BF16, tag="g1ring")       # D steps [m,s,b]

            nc.sync.dma_start(whh0_s[:], whh0_t[:])
            nc.sync.dma_start(b0_s[:], b0_t[:])
            nc.sync.dma_start(b1_s[:], b1_t[:])
            # initial h into the ring slots read at t=0 / j=0
            nc.sync.dma_start(h0ring[:, 15 * 512:16 * 512], h0i_t[:])
            nc.sync.dma_start(h1ring[:, 3 * 512:4 * 512], h1i_t[:])

            TH = 2048  # token half for XT chunking
            with tc.tile_pool(name="p0sb", bufs=2) as p0sb, \
                 tc.tile_pool(name="p0ev", bufs=2) as p0ev, \
                 tc.tile_pool(name="p0big", bufs=1) as p0big, \
                 tc.tile_pool(name="p0ps", bufs=2, space="PSUM") as p0ps:
                idxs = p0big.tile([128, ntok_pad // 128], I32, tag="idxs")
                nc.sync.dma_start(idxs[:], idx_t[:])
                zpad = p0big.tile([128, 512], BF16, tag="zpad")
                nc.gpsimd.memset(zpad[:], 0.0)
                nc.scalar.dma_start(
                    outs_d[ngrp - 1, :, :, 96:128].rearrange("k p b -> p k b"),
                    zpad[:].rearrange("p (k b) -> p k b", k=16))
                for it in range(ntok_pad // 128):
                    xg = p0sb.tile([128, H], F32, tag="xg")
                    nc.gpsimd.indirect_dma_start(
                        out=xg[:], out_offset=None, in_=emb_t[:],
                        in_offset=bass.IndirectOffsetOnAxis(
                            ap=idxs[:, it:it + 1], axis=0))
                    xc = p0sb.tile([128, H], BF16, tag="xc")
                    nc.vector.tensor_copy(xc[:], xg[:])
                    nc.scalar.dma_start(xbf_d[128 * it:128 * (it + 1), :], xc[:])

                wih0_s = p0big.tile([128, 8 * GS], BF16, tag="wih0s")
                nc.sync.dma_start(wih0_s[:], wih0_t[:])
                xt_s = p0big.tile([128, 8 * TH], BF16, tag="xts")

                for half in range((ntok + TH - 1) // TH):
                    t0 = TH * half
                    tw = min(TH, ntok - t0)
                    twp = ((tw + 15) // 16) * 16  # transpose src rows mult of 16
                    for k in range(8):
                        nc.sync.dma_start_transpose(
                            xt_s[:, TH * k:TH * k + twp],
                            xbf_d[t0:t0 + twp, 128 * k:128 * (k + 1)])
                    nch = [(512 * i, min(512, tw - 512 * i))
                           for i in range((tw + 511) // 512)]
                    for m in range(8):
                        ps = p0ps.tile([128, 2048], F32, tag="p0ps")
                        for k in range(8):
                            lhs = wih0_s[:, k * GS + 128 * m: k * GS + 128 * (m + 1)]
                            for (o, w) in nch:
                                nc.tensor.matmul(
                                    ps[:, o:o + w], lhs,
                                    xt_s[:, TH * k + o: TH * k + o + w],
                                    start=(k == 0), stop=(k == 7))
                        ev = p0ev.tile([128, TH], BF16, tag="g0ev")
                        nc.scalar.activation(ev[:, :tw], ps[:, :tw], AF.Identity,
                                             bias=b0_s[:, m:m + 1])
                        nc.scalar.dma_start(g0_d[m, :, t0:t0 + tw], ev[:, :tw])

            # ============ Phase 1: recurrence ============
            c_prev = [None, None]
            done_quarters = set()
            done_zb = set()
            done_pb = set()
            with tc.tile_pool(name="rp2", bufs=1) as rp2, \
                 tc.tile_pool(name="flp", bufs=1, space="PSUM") as flp, \
                 tc.tile_pool(name="gps", bufs=1, space="PSUM") as gps, \
                 tc.tile_pool(name="shp", bufs=1, space="PSUM") as shp, \
                 tc.tile_pool(name="gsbp", bufs=2) as gsbp, \
                 tc.tile_pool(name="posb", bufs=1) as posb_pool, \
                 tc.tile_pool(name="pscr", bufs=1) as pscr_pool, \
                 tc.tile_pool(name="pbp", bufs=2) as pbp, \
                 tc.tile_pool(name="zp", bufs=1) as zp, \
                 tc.tile_pool(name="cell", bufs=2) as cell_pool:

                wih1_s = rp2.tile([128, 16 * GS], FP8, tag="wih1s")
                whh1_s = rp2.tile([128, 16 * GS], FP8, tag="whh1s")
                nc.scalar.dma_start(wih1_s[:], wih1_t[:])
                nc.scalar.dma_start(whh1_s[:], whh1_t[:])
                nc.scalar.dma_start(wout_s[:], wout_t[:])

                def g0_prefetch(blk):
                    t0 = 8 * blk
                    nsteps = min(8, ts - t0)
                    if nsteps <= 0:
                        return
                    dst = g0ring[:].rearrange("p (h m s b) -> p h m s b",
                                              h=2, m=8, b=B)
                    src = g0_d[:, :, B * t0: B * (t0 + nsteps)].rearrange(
                        "m p sb -> p m sb")
                    nc.scalar.dma_start(
                        dst[:, blk % 2, :, 0:nsteps, :].rearrange(
                            "p m s b -> p m (s b)"), src)

                def flip_mms(w_s, h_of_kt, li):
                    """Flipped hh matmul into 4 one-bank psum tiles."""
                    Ps = [flp.tile([128, 512], F32, tag=f"fl{g}",
                                   name=f"fl{g}_{li}") for g in range(4)]
                    for r in range(8):
                        for g in range(4):
                            jm, jk = g >> 1, g & 1
                            kt = 8 * jk + r
                            nc.tensor.matmul(
                                Ps[g][32 * g:32 * (g + 1), :],
                                h_of_kt(kt),
                                w_s[:, kt * GS + 512 * jm: kt * GS + 512 * jm + 512],
                                start=(r == 0), stop=(r == 7),
                                tile_position=(0, 32 * g))
                    gsb = gsbp.tile([128, 1024], BF16, tag="gsb",
                                    name=f"gsb{li}")
                    for g in range(4):
                        jm = g >> 1
                        dst = gsb[32 * g:32 * (g + 1), 512 * jm:512 * jm + 512]
                        src = Ps[g][32 * g:32 * (g + 1), :]
                        if g % 2 == 0:
                            nc.scalar.activation(dst, src, AF.Identity)
                        else:
                            nc.vector.tensor_copy(dst, src)
                    return gsb

                def combine(gsb, li):
                    """Transpose-combine gsb blocks -> G [128q, 8m x 32b]."""
                    G = gps.tile([128, 512], F32, tag=f"G{li}", name=f"G{li}")
                    for cch in range(8):
                        jm = cch // 4
                        nc.tensor.matmul(
                            G[:, B * cch:B * (cch + 1)],
                            gsb[64 * jm:64 * jm + 64, 128 * cch:128 * (cch + 1)],
                            s_s[64 * jm:64 * jm + 64, :],
                            start=True, stop=True)
                    return G

                def cell(l, G, gadd_ap):
                    """LSTM cell for layer l; returns hn tile [128, 64] bf16."""
                    g = cell_pool.tile([128, 256], F32, tag=f"g{l}")
                    nc.vector.tensor_add(
                        g[:].rearrange("p (m b) -> p m b", b=B),
                        G[:, :256].rearrange("p (m b) -> p m b", b=B),
                        gadd_ap)
                    sfo = cell_pool.tile([128, 192], F32, tag=f"sfo{l}")
                    nc.scalar.activation(sfo[:], g[:, 0:192], AF.Sigmoid)
                    tg = cell_pool.tile([128, 64], F32, tag=f"tg{l}")
                    nc.scalar.activation(tg[:], g[:, 192:256], AF.Tanh)
                    t1 = cell_pool.tile([128, 64], F32, tag=f"t1{l}")
                    nc.vector.tensor_mul(t1[:], sfo[:, 0:64], tg[:])
                    t2 = cell_pool.tile([128, 64], F32, tag=f"t2{l}")
                    nc.vector.tensor_mul(t2[:], sfo[:, 64:128], c_prev[l][:])
                    cn = cell_pool.tile([128, 64], F32, tag=f"cn{l}")
                    nc.vector.tensor_add(cn[:], t1[:], t2[:])
                    c_prev[l] = cn
                    tcn = cell_pool.tile([128, 64], F32, tag=f"tc{l}")
                    nc.scalar.activation(tcn[:], cn[:], AF.Tanh)
                    hn = cell_pool.tile([128, 64], BF16, tag=f"hn{l}")
                    nc.vector.tensor_mul(hn[:], sfo[:, 128:192], tcn[:])
                    return hn

                def zbatch(b):
                    """z-AllReduce + logZ for groups 4b..4b+3."""
                    gsl = slice(4 * b, 4 * b + 4)
                    m4v = m4[:].rearrange("p (g q) -> p g q", q=4)[:, gsl, :]
                    s4v = s4[:].rearrange("p (g q) -> p g q", q=4)[:, gsl, :]
                    t01 = zp.tile([128, 4], F32, tag="t01", name=f"t01_{b}")
                    t23 = zp.tile([128, 4], F32, tag="t23", name=f"t23_{b}")
                    nc.vector.tensor_max(t01[:], m4v[:, :, 0], m4v[:, :, 1])
                    nc.vector.tensor_max(t23[:], m4v[:, :, 2], m4v[:, :, 3])
                    mall = zp.tile([128, 4], F32, tag="mall", name=f"ma{b}")
                    nc.vector.tensor_max(mall[:], t01[:], t23[:])
                    acc = zp.tile([128, 4], F32, tag="acc", name=f"ac{b}")
                    dq = zp.tile([128, 4], F32, tag="dq", name=f"dq{b}")
                    for q in range(4):
                        nc.vector.tensor_sub(dq[:], m4v[:, :, q], mall[:])
                        nc.scalar.activation(dq[:], dq[:], AF.Exp, scale=1.0 / OS)
                        nc.vector.tensor_mul(dq[:], dq[:], s4v[:, :, q])
                        if q == 0:
                            nc.vector.tensor_copy(acc[:], dq[:])
                        else:
                            nc.vector.tensor_add(acc[:], acc[:], dq[:])
                    em = zp.tile([128, 4], F32, tag="em", name=f"em{b}")
                    nc.scalar.activation(em[:], mall[:], AF.Exp, scale=1.0 / OS)
                    zt = zp.tile([128, 4], F32, tag="zt", name=f"zt{b}")
                    nc.vector.tensor_mul(zt[:], acc[:], em[:])
                    zloc = agp.tile([128, 4], F32, tag="zloc", name=f"zl{b}")
                    zglob = agp.tile([128, 4], F32, tag="zglob",
                                     name=f"zg{b}", addr_space="Shared")
                    nc.sync.dma_start(zloc[:], zt[:])
                    nc.gpsimd.collective_compute(
                        "AllReduce", mybir.AluOpType.add, replica_groups=RG,
                        ins=[zloc[:].opt()], outs=[zglob[:].opt()])
                    zg_s = zp.tile([128, 4], F32, tag="zgs", name=f"zs{b}")
                    nc.sync.dma_start(zg_s[:], zglob[:])
                    nc.scalar.activation(logZ[:, gsl], zg_s[:], AF.Ln)
                    done_zb.add(b)

                def passb_chunk(g, ci):
                    """logp chunk: out[128g.., 1000ci..] = lin/OS - logZ[g]."""
                    v0 = VQ * ci
                    lin = pbp.tile([128, VQ], BF16, tag="lin",
                                   name=f"li{g}_{ci}")
                    nc.sync.dma_start(lin[:], logits_d[g, :, v0:v0 + VQ])
                    lout = pbp.tile([128, VQ], BF16, tag="lout",
                                    name=f"lo{g}_{ci}")
                    nc.vector.tensor_scalar(lout[:], lin[:], 1.0 / OS,
                                            logZ[:, g:g + 1],
                                            op0=mybir.AluOpType.mult,
                                            op1=mybir.AluOpType.subtract)
                    nc.sync.dma_start(
                        out_t[128 * g:128 * (g + 1), v0:v0 + VQ], lout[:])
                    done_pb.add((g, ci))

                c0s = cell_pool.tile([128, 64], F32, tag="cn0")
                nc.sync.dma_start(c0s[:], c0i_t[:])
                c_prev[0] = c0s
                c1s = cell_pool.tile([128, 64], F32, tag="cn1")
                nc.sync.dma_start(c1s[:], c1i_t[:])
                c_prev[1] = c1s

                g0_prefetch(0)
                g0_prefetch(1)

                h0r4 = h0ring[:].rearrange("p (s k b) -> p s k b", s=16, b=B)
                h1r4 = h1ring[:].rearrange("p (s k b) -> p s k b", s=4, b=B)
                g0r5 = g0ring[:].rearrange("p (h m s b) -> p h m s b",
                                           h=2, m=8, b=B)
                g1r4 = g1ring[:].rearrange("p (m s b) -> p m s b", m=8, b=B)

                # schedules: one projection quarter per tick; z-AR per 4
                # groups; one passB chunk per tick after logZ is known
                psched = {}
                if PROJ_INREC:
                    for g in range(ngrp - 1):
                        for q in range(4):
                            t_q = 4 * g + 14 + q
                            if t_q < nticks:
                                psched[t_q] = (g, q)
                zsched = {}
                pbsched = {}
                if INREC_Z:
                    for b2 in range((ngrp + 3) // 4):
                        tz = 16 * b2 + 31
                        if tz < nticks:
                            zsched[tz] = b2
                        for i in range(16):
                            tp = 16 * b2 + 32 + i
                            if tp < nticks:
                                pbsched[tp] = (4 * b2 + i // 4, i % 4)
                cur_osb = [None]

                # split per-layer AllGathers: each is triggered right after
                # its cell and consumed ~a full tick later, so the ~10us
                # collective+DMA chain stays off the critical path.
                for t in range(nticks):
                    j = t - LAG  # layer-1 step this tick

                    pj = psched.get(t)
                    psq = None
                    if pj is not None:
                        g, q = pj

                    # ---- layer 0, step t ----
                    if t < ts:
                        gsb0 = flip_mms(
                            whh0_s,
                            lambda kt: h0r4[:, (t - 1) % 16, kt, :], "a")
                        G0 = combine(gsb0, "a")
                        if t % 8 == 7:
                            g0_prefetch(t // 8 + 2)
                        hn0 = cell(0, G0, g0r5[:, (t // 8) % 2, :, t % 8, :])
                        agin0 = agp.tile([2, 128, B], BF16, tag="agin0",
                                         name=f"ai0_{t}")
                        nc.sync.dma_start(
                            agin0[:].rearrange("j p b -> p j b"),
                            hn0[:].rearrange("p (j b) -> p j b", b=B))
                        agout0 = agp.tile([NC, 2, 128, B], BF16, tag="agout0",
                                          name=f"ao0_{t}", addr_space="Shared")
                        nc.gpsimd.collective_compute(
                            "AllGather", mybir.AluOpType.bypass,
                            replica_groups=RG,
                            ins=[agin0[:].opt()], outs=[agout0[:].opt()])
                        nc.sync.dma_start(
                            h0r4[:, t % 16, :, :],
                            agout0[:].rearrange("r j p b -> p (r j) b"))

                    # ---- projection quarter (fills the AG flight window) ----
                    if pj is not None:
                        if q == 0:
                            osb = posb_pool.tile([128, 2048], BF16, tag="posb",
                                                 name=f"osb{g}")
                            nc.sync.dma_start(
                                osb[:].rearrange("p (k q) -> p k q", k=16),
                                outs_d[g, :, :, :].rearrange("k p q -> p k q"))
                            cur_osb[0] = osb
                        psq = shp.tile([128, 1024], F32, tag="sh",
                                       name=f"psq{g}_{q}")
                        proj_mms(g, q, cur_osb[0], psq, range(16), True)

                    # ---- layer-1 input pass every D steps ----
                    if 0 <= j < ts and j % D == 0:
                        nb = min(D, ts - j)
                        s0 = j % 16
                        for qi in range(4):
                            psg = flp.tile([128, 512], F32, tag=f"fl{qi}",
                                           name=f"psg{t}_{qi}")
                            for lmi in range(2):
                                m = 2 * qi + lmi
                                for k in range(16):
                                    nc.tensor.matmul(
                                        psg[:, 256 * lmi: 256 * lmi + B * nb],
                                        wih1_s[:, k * GS + 128 * m:
                                               k * GS + 128 * (m + 1)],
                                        h0r4[:, s0:s0 + nb, k, :],
                                        start=(k == 0), stop=(k == 15))
                            for lmi in range(2):
                                m = 2 * qi + lmi
                                nc.scalar.activation(
                                    g1ring[:, 256 * m: 256 * m + B * nb],
                                    psg[:, 256 * lmi: 256 * lmi + B * nb],
                                    AF.Identity, scale=1.0 / OS,
                                    bias=b1_s[:, m:m + 1])

                    # ---- layer 1, step j ----
                    if 0 <= j < ts:
                        gsb1 = flip_mms(
                            whh1_s,
                            lambda kt: h1r4[:, (j - 1) % 4, kt, :], "b")
                        G1 = combine(gsb1, "b")
                        hn1 = cell(1, G1, g1r4[:, :, j % D, :])
                        agin1 = agp.tile([2, 128, B], BF16, tag="agin1",
                                         name=f"ai1_{t}")
                        nc.sync.dma_start(
                            agin1[:].rearrange("j p b -> p j b"),
                            hn1[:].rearrange("p (j b) -> p j b", b=B))
                        agout1 = agp.tile([NC, 2, 128, B], BF16, tag="agout1",
                                          name=f"ao1_{t}", addr_space="Shared")
                        nc.gpsimd.collective_compute(
                            "AllGather", mybir.AluOpType.bypass,
                            replica_groups=RG,
                            ins=[agin1[:].opt()], outs=[agout1[:].opt()])
                        nc.sync.dma_start(
                            h1r4[:, j % 4, :, :],
                            agout1[:].rearrange("r j p b -> p (r j) b"))
                        nc.scalar.dma_start(
                            outs_d[j // 4, :, :, B * (j % 4):B * (j % 4 + 1)]
                            .rearrange("k p b -> p k b"),
                            h1r4[:, j % 4, :, :])

                    # stats at tick end (keeps DVE/ACT off the PE's path)
                    if pj is not None:
                        proj_stats(g, q, psq, pscr_pool)
                        done_quarters.add((g, q))
                    if t in zsched:
                        zbatch(zsched[t])
                    if t in pbsched:
                        gpb, ci = pbsched[t]
                        passb_chunk(gpb, ci)

        # ============ Phase 2 tail: leftovers ============
        with tc.tile_pool(name="p2sb", bufs=2) as p2sb, \
             tc.tile_pool(name="p2scr", bufs=2) as p2scr, \
             tc.tile_pool(name="p2z", bufs=1) as p2z, \
             tc.tile_pool(name="p2pb", bufs=2) as p2pb, \
             tc.tile_pool(name="p2ps", bufs=2, space="PSUM") as p2ps:

            # remaining projection quarters
            for g in range(ngrp):
                rem = [q for q in range(4) if (g, q) not in done_quarters]
                if not rem:
                    continue
                osb = p2sb.tile([128, 2048], BF16, tag="osb", name=f"osbt{g}")
                nc.sync.dma_start(
                    osb[:].rearrange("p (k q) -> p k q", k=16),
                    outs_d[g, :, :, :].rearrange("k p q -> p k q"))
                for q in rem:
                    psq = p2ps.tile([128, 1024], F32, tag="sh",
                                    name=f"tps{g}_{q}")
                    proj_mms(g, q, osb, psq, range(16), True)
                    proj_stats(g, q, psq, p2scr)

            # remaining z batches (reuse the in-rec helper pools shapes)
            nb2 = (ngrp + 3) // 4
            for b2 in range(nb2):
                if b2 in done_zb:
                    continue
                gsl = slice(4 * b2, 4 * b2 + 4)
                m4v = m4[:].rearrange("p (g q) -> p g q", q=4)[:, gsl, :]
                s4v = s4[:].rearrange("p (g q) -> p g q", q=4)[:, gsl, :]
                t01 = p2z.tile([128, 4], F32, tag="t01", name=f"u01_{b2}")
                t23 = p2z.tile([128, 4], F32, tag="t23", name=f"u23_{b2}")
                nc.vector.tensor_max(t01[:], m4v[:, :, 0], m4v[:, :, 1])
                nc.vector.tensor_max(t23[:], m4v[:, :, 2], m4v[:, :, 3])
                mall = p2z.tile([128, 4], F32, tag="mall", name=f"uma{b2}")
                nc.vector.tensor_max(mall[:], t01[:], t23[:])
                acc = p2z.tile([128, 4], F32, tag="acc", name=f"uac{b2}")
                dq = p2z.tile([128, 4], F32, tag="dq", name=f"udq{b2}")
                for q in range(4):
                    nc.vector.tensor_sub(dq[:], m4v[:, :, q], mall[:])
                    nc.scalar.activation(dq[:], dq[:], AF.Exp, scale=1.0 / OS)
                    nc.vector.tensor_mul(dq[:], dq[:], s4v[:, :, q])
                    if q == 0:
                        nc.vector.tensor_copy(acc[:], dq[:])
                    else:
                        nc.vector.tensor_add(acc[:], acc[:], dq[:])
                em = p2z.tile([128, 4], F32, tag="em", name=f"uem{b2}")
                nc.scalar.activation(em[:], mall[:], AF.Exp, scale=1.0 / OS)
                zt = p2z.tile([128, 4], F32, tag="zt", name=f"uzt{b2}")
                nc.vector.tensor_mul(zt[:], acc[:], em[:])
                zloc = agp.tile([128, 4], F32, tag="zloc", name=f"uzl{b2}")
                zglob = agp.tile([128, 4], F32, tag="zglob",
                                 name=f"uzg{b2}", addr_space="Shared")
                nc.sync.dma_start(zloc[:], zt[:])
                nc.gpsimd.collective_compute(
                    "AllReduce", mybir.AluOpType.add, replica_groups=RG,
                    ins=[zloc[:].opt()], outs=[zglob[:].opt()])
                zg_s = p2z.tile([128, 4], F32, tag="zgs", name=f"uzs{b2}")
                nc.sync.dma_start(zg_s[:], zglob[:])
                nc.scalar.activation(logZ[:, gsl], zg_s[:], AF.Ln)

            # remaining passB chunks
            for g in range(ngrp):
                for ci in range(4):
                    if (g, ci) in done_pb:
                        continue
                    v0 = VQ * ci
                    lin = p2pb.tile([128, VQ], BF16, tag="lin",
                                    name=f"uli{g}_{ci}")
                    nc.sync.dma_start(lin[:], logits_d[g, :, v0:v0 + VQ])
                    lout = p2pb.tile([128, VQ], BF16, tag="lout",
                                     name=f"ulo{g}_{ci}")
                    nc.vector.tensor_scalar(lout[:], lin[:], 1.0 / OS,
                                            logZ[:, g:g + 1],
                                            op0=mybir.AluOpType.mult,
                                            op1=mybir.AluOpType.subtract)
                    nc.sync.dma_start(
                        out_t[128 * g:128 * (g + 1), v0:v0 + VQ], lout[:])

    nc.compile()
    return nc


_NC_CACHE = {}


def _get_nc(ts):
    if ts not in _NC_CACHE:
        _NC_CACHE[ts] = build_nc(ts)
    return _NC_CACHE[ts]


def run_device(inputs, ts=TS_FULL, **run_kwargs):
    in_maps, ntok_pad = prep_inputs(inputs, ts)
    nc = _get_nc(ts)
    res = bass_utils.run_bass_kernel_spmd(nc, in_maps,
                                          core_ids=list(range(NC)), **run_kwargs)
    ntok = ts * B
    logp = np.empty((ntok, V), np.float32)
    for c in range(NC):
        logp[:, VS * c:VS * (c + 1)] = res.results[c]["out"][:ntok].astype(
            np.float32)
    out = np.zeros((B, T, V), np.float32)
    out[:, 0, SOS_ID] = 1.0
    out[:, 1:1 + ts, :] = logp.reshape(ts, B, V).transpose(1, 0, 2)
    return out, res


def kernel(**inputs) -> np.ndarray:
    out, _ = run_device(inputs, TS_FULL)
    return out
